# revision 1
# baseline (speedup 1.0000x reference)
# Trainium2 Bass kernel for the 2-layer GNN message-passing block.
# Self-contained: hardcodes shapes; takes full inputs, shards across 8 cores,
# returns the full [50000, 128] float32 output.
import os
import sys

sys.path.insert(0, "/opt/trn_rl_repo")

import numpy as np
import ml_dtypes

import concourse.bacc as bacc
import concourse.tile as tile
from concourse import mybir
from concourse.bass_utils import run_bass_kernel_spmd

BF16 = ml_dtypes.bfloat16

N = 50000
NPAD = 50176
NC = 8
C = NPAD // NC            # 6272 nodes per core
WCNT = C // 128           # 49 windows of 128 nodes
HALF = NPAD // 2          # 25088 (int16-addressable gather halves)
GOP = int(os.environ.get("KERNEL_GOP", "1024"))  # edges per dma_gather op
SBC = 8                   # chunks (of 128 edges) per compute sub-batch

F32 = mybir.dt.float32
BF = mybir.dt.bfloat16
I16 = mybir.dt.int16


def _bf(x):
    return np.ascontiguousarray(x.astype(BF16))


def _prep(inputs):
    """Host-side graph partitioning. Returns per-core input dicts + metadata."""
    src = np.asarray(inputs["edge_index"][0]).astype(np.int64)
    dst = np.asarray(inputs["edge_index"][1]).astype(np.int64)
    ef = np.asarray(inputs["edge_features"]).astype(np.float32)
    E = src.shape[0]

    owner = dst // C
    halfe = src // HALF
    dl = dst - owner * C
    win = dl // 128
    wl = dl % 128

    key = (owner * 2 + halfe) * WCNT + win
    order = np.argsort(key, kind="stable")
    ksort = key[order]
    counts_flat = np.bincount(key, minlength=NC * 2 * WCNT)
    counts = counts_flat.reshape(NC, 2, WCNT)

    nch = np.maximum(1, -(-counts.max(axis=0) // 128)).astype(np.int64)  # [2,WCNT]
    seg_len = nch * 128
    L0s = int(seg_len[0].sum())
    L1s = int(seg_len[1].sum())
    pad0 = (-L0s) % GOP
    L0p = L0s + pad0
    pad1 = (-L1s) % GOP
    L = L0p + L1s + pad1

    seg_start = np.zeros((2, WCNT), np.int64)
    pos = 0
    for w in range(WCNT):
        seg_start[0, w] = pos
        pos += seg_len[0, w]
    pos = L0p
    for w in range(WCNT):
        seg_start[1, w] = pos
        pos += seg_len[1, w]

    # destination position of each edge within its core's stream
    group_first = np.cumsum(counts_flat) - counts_flat
    within = np.arange(E, dtype=np.int64) - group_first[ksort]
    dest = seg_start[halfe[order], win[order]] + within

    owner_s = owner[order]

    # chunk metadata: (window, is_first, is_last, real?) per 128-edge chunk
    nchunks = L // 128
    chunk_meta = []
    cw = np.full(nchunks, -1, np.int64)
    cf = np.zeros(nchunks, bool)
    cl = np.zeros(nchunks, bool)
    ch = np.zeros(nchunks, np.int64)
    for h in range(2):
        for w in range(WCNT):
            s = int(seg_start[h, w]) // 128
            n = int(nch[h, w])
            cw[s:s + n] = w
            cf[s] = True
            cl[s + n - 1] = True
            ch[s:s + n] = h
    for cc in range(nchunks):
        chunk_meta.append((int(cw[cc]), bool(cf[cc]), bool(cl[cc]), int(ch[cc]),
                           cw[cc] >= 0))

    x = np.asarray(inputs["x"]).astype(np.float32)
    xpad = np.zeros((NPAD, 128), np.float32)
    xpad[:N] = x
    xT_bf = _bf(xpad.T)

    iota = np.tile(np.arange(128, dtype=np.float32)[None, :], (128, 1))
    ident = np.eye(128, dtype=np.float32)

    shared = {
        "xT": xT_bf,
        "W1": _bf(np.asarray(inputs["ff1_W"], np.float32)),
        "Ws1": _bf(np.asarray(inputs["mp1_Wsrc"], np.float32)),
        "Wd1": _bf(np.asarray(inputs["mp1_Wdst"], np.float32)),
        "We1": _bf(np.asarray(inputs["mp1_We"], np.float32)),
        "Ws2": _bf(np.asarray(inputs["mp2_Wsrc"], np.float32)),
        "Wd2": _bf(np.asarray(inputs["mp2_Wdst"], np.float32)),
        "We2": _bf(np.asarray(inputs["mp2_We"], np.float32)),
        "W3": _bf(np.asarray(inputs["ff2_W"], np.float32)),
        "b1c": np.ascontiguousarray(
            np.asarray(inputs["ff1_b"], np.float32)[:, None]),
        "b1m": np.ascontiguousarray(
            np.tile(np.asarray(inputs["mp1_b"], np.float32)[None, :], (128, 1))),
        "b2m": np.ascontiguousarray(
            np.tile(np.asarray(inputs["mp2_b"], np.float32)[None, :], (128, 1))),
        "b3m": np.ascontiguousarray(
            np.tile(np.asarray(inputs["ff2_b"], np.float32)[None, :], (128, 1))),
        "iota": _bf(iota),
        "iotac": _bf(np.arange(128, dtype=np.float32)[:, None]),
        "onesb": _bf(np.ones((1, 128), np.float32)),
        "identb": _bf(ident),
        "identf": ident,
    }

    per_core = []
    for c in range(NC):
        m = owner_s == c
        e_ids = order[m]
        dp = dest[m]
        eT = np.zeros((64, L), np.float32)
        eT[:, dp] = ef[e_ids].T
        edc = np.full(L, -1.0, np.float32)
        edc[dp] = wl[e_ids]
        sidx = np.zeros(L, np.int16)
        sidx[dp] = (src[e_ids] - halfe[e_ids] * HALF).astype(np.int16)
        didx = np.zeros(L, np.int16)
        didx[dp] = (dst[e_ids] - c * C).astype(np.int16)

        # wrap layouts
        edc_cw = np.ascontiguousarray(edc.reshape(L // 128, 128).T)     # [128, L/128]
        s_wr = np.ascontiguousarray(
            np.tile(sidx.reshape(L // 16, 16).T, (8, 1)))               # [128, L/16]

        per_core.append({
            "eT": _bf(eT),
            "edc": _bf(edc_cw),
            "edr": _bf(edc[None, :]),
            "srcw": s_wr,
            "xoT": _bf(xpad[c * C:(c + 1) * C].T),
        })
    meta = dict(L=L, L0p=L0p, chunk_meta=chunk_meta)
    return shared, per_core, meta


def _build(L, L0p, chunk_meta):
    """Build the SPMD Bass program (identical for all 8 cores)."""
    nc = bacc.Bacc("TRN2", target_bir_lowering=False, debug=False, num_devices=NC,
                   num_swdge_queues=4,
                   dynamic_dma_scratch_size=int(os.environ.get("KERNEL_DDS", "16384")))
    GELU = (mybir.ActivationFunctionType.Identity
            if os.environ.get("KERNEL_SIM_IDENTITY") == "1"
            else mybir.ActivationFunctionType.Gelu_apprx_tanh)
    EQ = mybir.AluOpType.is_equal

    # I/O
    t_xT = nc.dram_tensor("xT", [128, NPAD], BF, kind="ExternalInput")
    t_xoT = nc.dram_tensor("xoT", [128, C], BF, kind="ExternalInput")
    t_eT = nc.dram_tensor("eT", [64, L], BF, kind="ExternalInput")
    t_edc = nc.dram_tensor("edc", [128, L // 128], BF, kind="ExternalInput")
    t_srcw = nc.dram_tensor("srcw", [128, L // 16], I16, kind="ExternalInput")
    t_edr = nc.dram_tensor("edr", [1, L], BF, kind="ExternalInput")
    wts = {}
    for nm, shape, dt in [
        ("W1", [128, 128], BF), ("Ws1", [128, 128], BF), ("Wd1", [128, 128], BF),
        ("We1", [64, 128], BF), ("Ws2", [128, 128], BF), ("Wd2", [128, 128], BF),
        ("We2", [64, 128], BF), ("W3", [128, 128], BF),
        ("b1c", [128, 1], F32), ("b1m", [128, 128], F32), ("b2m", [128, 128], F32),
        ("b3m", [128, 128], F32), ("iota", [128, 128], BF),
        ("iotac", [128, 1], BF), ("onesb", [1, 128], BF),
        ("identb", [128, 128], BF), ("identf", [128, 128], F32),
    ]:
        wts[nm] = nc.dram_tensor(nm, shape, dt, kind="ExternalInput")
    t_out = nc.dram_tensor("out", [C, 128], F32, kind="ExternalOutput")

    NOPS = L // GOP
    NCH = L // 128

    with tile.TileContext(nc) as tc:
        with (
            tc.tile_pool(name="persist", bufs=1) as pp,
            tc.tile_pool(name="dram", bufs=1, space="DRAM") as dram,
        ):
            # persistent SBUF state
            wt = {}
            for nm in ["W1", "Ws1", "Wd1", "We1", "Ws2", "Wd2", "We2", "W3",
                       "b1c", "b1m", "b2m", "b3m", "iota", "iotac", "onesb", "identb",
                       "identf"]:
                shape = wts[nm].shape
                dt = {"b1c": F32, "b1m": F32, "b2m": F32, "b3m": F32,
                      "identf": F32}.get(nm, BF)
                wt[nm] = pp.tile(list(shape), dt, tag=f"w_{nm}", name=f"w_{nm}")
                nc.sync.dma_start(out=wt[nm][:], in_=wts[nm][:])
            edc_t = pp.tile([128, NCH], BF, tag="edc")
            nc.sync.dma_start(out=edc_t[:], in_=t_edc[:])
            srcw_t = pp.tile([128, L // 16], I16, tag="srcw")
            nc.sync.dma_start(out=srcw_t[:], in_=t_srcw[:])
            h_own = pp.tile([128, C], BF, tag="h_own")
            xd_sb = pp.tile([128, C], BF, tag="xd_sb")      # node-major own windows
            agg_sb = pp.tile([128, C], F32, tag="agg_sb")   # per-window agg (half 0)

            # internal DRAM
            xs_d = [dram.tile([NPAD, 128], BF, tag=f"xs{l}", name=f"xs{l}") for l in range(2)]
            ag_in = dram.tile([C, 128], BF, tag="ag_in")
            ag_out = dram.tile([NPAD, 128], BF, tag="ag_out", addr_space="Shared")

            r4096 = nc.gpsimd.to_reg(GOP)

            # ---------------- dense phase ----------------
            def dense_full(layer):
                """xs[layer] for all NPAD nodes."""
                Ws = wt["Ws1"] if layer == 0 else wt["Ws2"]
                with (
                    tc.tile_pool(name=f"dA{layer}", bufs=3) as dp,
                    tc.tile_pool(name=f"dAp{layer}", bufs=2, space="PSUM") as dq,
                ):
                    for g in range(NPAD // 512):
                        hT = dp.tile([128, 512], BF, tag="hT")
                        if layer == 0:
                            xt = dp.tile([128, 512], BF, tag="xt")
                            nc.sync.dma_start(
                                out=xt[:], in_=t_xT[:, g * 512:(g + 1) * 512])
                            ps = dq.tile([128, 512], F32, tag="ps")
                            nc.tensor.matmul(out=ps[:], lhsT=wt["W1"][:],
                                             rhs=xt[:], start=True, stop=True)
                            nc.scalar.activation(out=hT[:], in_=ps[:], func=GELU,
                                                 bias=wt["b1c"][:])
                        else:
                            nc.sync.dma_start(
                                out=hT[:],
                                in_=ag_out[g * 512:(g + 1) * 512, :],
                                transpose=True)
                        px = dq.tile([128, 4, 128], F32, tag="px")
                        for j in range(4):
                            nc.tensor.matmul(out=px[:, j, :],
                                             lhsT=hT[:, j * 128:(j + 1) * 128],
                                             rhs=Ws[:], start=True, stop=True)
                        xs_sb = dp.tile([128, 4, 128], BF, tag="xs_sb")
                        nc.vector.tensor_copy(out=xs_sb[:], in_=px[:])
                        nc.sync.dma_start(
                            out=xs_d[layer][g * 512:(g + 1) * 512, :]
                                .rearrange("(j p) f -> p j f", p=128),
                            in_=xs_sb[:])

            def dense_own_l0():
                """h0 own (node-major) + xd0+b for own nodes, from x_own_T."""
                with (
                    tc.tile_pool(name="dB", bufs=3) as dp,
                    tc.tile_pool(name="dBp", bufs=2, space="PSUM") as dq,
                ):
                    ngrp = (C + 511) // 512
                    for g in range(ngrp):
                        c0 = g * 512
                        cn = min(512, C - c0)
                        xt = dp.tile([128, 512], BF, tag="xt")
                        nc.sync.dma_start(out=xt[:, :cn], in_=t_xoT[:, c0:c0 + cn])
                        ps = dq.tile([128, 512], F32, tag="ps")
                        nc.tensor.matmul(out=ps[:, :cn], lhsT=wt["W1"][:],
                                         rhs=xt[:, :cn], start=True, stop=True)
                        hT = dp.tile([128, 512], BF, tag="hT")
                        nc.scalar.activation(out=hT[:, :cn], in_=ps[:, :cn],
                                             func=GELU, bias=wt["b1c"][:])
                        for j in range(cn // 128):
                            lw = c0 + j * 128
                            sl = hT[:, j * 128:(j + 1) * 128]
                            pn = dq.tile([128, 128], F32, tag="pn")
                            nc.tensor.matmul(out=pn[:], lhsT=sl, rhs=wt["identb"][:],
                                             start=True, stop=True)
                            nc.vector.tensor_copy(out=h_own[:, lw:lw + 128], in_=pn[:])
                            pd = dq.tile([128, 128], F32, tag="pd")
                            nc.tensor.matmul(out=pd[:], lhsT=sl, rhs=wt["Wd1"][:],
                                             start=True, stop=True)
                            nc.vector.tensor_add(out=xd_sb[:, lw:lw + 128],
                                                 in0=pd[:], in1=wt["b1m"][:])

            def dense_own_l1():
                """xd1+b for own nodes from h_own (h1, node-major in SBUF)."""
                with (
                    tc.tile_pool(name="dC", bufs=3) as dp,
                    tc.tile_pool(name="dCp", bufs=2, space="PSUM") as dq,
                ):
                    for w in range(WCNT):
                        ws = slice(w * 128, (w + 1) * 128)
                        pt = dq.tile([128, 128], BF, tag="pt")
                        nc.tensor.transpose(out=pt[:], in_=h_own[:, ws],
                                            identity=wt["identb"][:])
                        hT = dp.tile([128, 128], BF, tag="hT")
                        nc.vector.tensor_copy(out=hT[:], in_=pt[:])
                        pd = dq.tile([128, 128], F32, tag="pd")
                        nc.tensor.matmul(out=pd[:], lhsT=hT[:], rhs=wt["Wd2"][:],
                                         start=True, stop=True)
                        nc.vector.tensor_add(out=xd_sb[:, w * 128:(w + 1) * 128],
                                             in0=pd[:], in1=wt["b2m"][:])

            # ---------------- edge phase ----------------
            def edge_phase(layer):
                We = wt["We1"] if layer == 0 else wt["We2"]
                with (
                    tc.tile_pool(name=f"eS{layer}", bufs=3) as ep,
                    tc.tile_pool(name=f"eG{layer}", bufs=6) as gp,
                    tc.tile_pool(name=f"eP{layer}", bufs=2, space="PSUM") as qp,
                    tc.tile_pool(name=f"eA{layer}", bufs=2, space="PSUM") as ap_,
                    tc.tile_pool(name=f"eF{layer}", bufs=2) as fp,
                ):
                    active_agg = [None]

                    def finalize(h, w, agg_ps):
                        ws = slice(w * 128, (w + 1) * 128)
                        if h == 0:
                            nc.vector.tensor_copy(out=agg_sb[:, ws], in_=agg_ps[:])
                            return
                        t1 = fp.tile([128, 128], F32, tag="f1")
                        nc.vector.tensor_add(out=t1[:], in0=agg_ps[:],
                                             in1=agg_sb[:, ws])
                        if layer == 0:
                            h1w = fp.tile([128, 128], BF, tag="fh")
                            nc.vector.tensor_add(out=h1w[:], in0=t1[:],
                                                 in1=h_own[:, ws])
                            nc.vector.tensor_copy(out=h_own[:, ws], in_=h1w[:])
                            nc.sync.dma_start(out=ag_in[w * 128:(w + 1) * 128, :],
                                              in_=h1w[:])
                        else:
                            h2w = fp.tile([128, 128], F32, tag="fh2")
                            nc.vector.tensor_add(out=h2w[:], in0=t1[:],
                                                 in1=h_own[:, ws])
                            pt = ap_.tile([128, 128], F32, tag="tp", bufs=1)
                            nc.tensor.transpose(out=pt[:], in_=h2w[:],
                                                identity=wt["identf"][:])
                            h2T = fp.tile([128, 128], BF, tag="h2T")
                            nc.vector.tensor_copy(out=h2T[:], in_=pt[:])
                            po = ap_.tile([128, 128], F32, tag="tp", bufs=1,
                                          name="po")
                            nc.tensor.matmul(out=po[:], lhsT=h2T[:], rhs=wt["W3"][:],
                                             start=True, stop=True)
                            osb = fp.tile([128, 128], F32, tag="osb")
                            nc.vector.tensor_add(out=osb[:], in0=po[:],
                                                 in1=wt["b3m"][:])
                            nc.sync.dma_start(out=t_out[w * 128:(w + 1) * 128, :],
                                              in_=osb[:])

                    for op_i in range(NOPS):
                        e0 = op_i * GOP
                        cc0 = e0 // 128
                        nreal = sum(1 for k in range(GOP // 128) if chunk_meta[cc0 + k][4])
                        if nreal == 0:
                            continue
                        half = 0 if e0 < L0p else 1
                        src_ap = xs_d[layer][half * HALF:(half + 1) * HALF, :]
                        xsg = gp.tile([128, GOP // 128, 128], BF, tag="xsg")
                        nc.gpsimd.dma_gather(
                            xsg[:], src_ap, srcw_t[:, e0 // 16:(e0 + GOP) // 16],
                            GOP, r4096, 128, elem_step=128,
                            queue_num=op_i % 4)
                        eTt = ep.tile([64, GOP], BF, tag="eTt")
                        nc.sync.dma_start(out=eTt[:], in_=t_eT[:, e0:e0 + GOP])
                        edrs = ep.tile([1, GOP], BF, tag="edrs")
                        nc.sync.dma_start(out=edrs[:], in_=t_edr[0:1, e0:e0 + GOP])

                        for sb0 in range(0, nreal, SBC):
                            nb = min(SBC, nreal - sb0)
                            psq = qp.tile([128, SBC, 128], F32, tag="psq")
                            Pt = ep.tile([128, SBC, 128], BF, tag="Pt")
                            for h2 in range(0, nb, 4):
                                hb = min(4, nb - h2)
                                eb = ap_.tile([128, 4, 128], F32, tag="eb", bufs=1,
                                              name="eb")
                                nc.tensor.matmul(
                                    out=eb[:].rearrange("p b i -> p (b i)")[:, :hb * 128],
                                    lhsT=wt["onesb"][:],
                                    rhs=edrs[0:1, (sb0 + h2) * 128:
                                             (sb0 + h2 + hb) * 128],
                                    start=True, stop=True)
                                nc.vector.tensor_tensor(
                                    out=Pt[:, h2:h2 + hb, :],
                                    in0=eb[:, :hb, :],
                                    in1=wt["iotac"][:, None, :]
                                        .to_broadcast([128, hb, 128]),
                                    op=EQ)
                            for j in range(nb):
                                cl = sb0 + j
                                w_j = chunk_meta[cc0 + cl][0]
                                ws_j = slice(w_j * 128, (w_j + 1) * 128)
                                nc.tensor.matmul(
                                    out=psq[:, j, :],
                                    lhsT=eTt[:, cl * 128:(cl + 1) * 128],
                                    rhs=We[:], start=True, stop=False)
                                nc.tensor.matmul(
                                    out=psq[:, j, :], lhsT=Pt[:, j, :],
                                    rhs=xd_sb[:, ws_j], start=False, stop=True)
                            qsb = ep.tile([128, SBC, 128], F32, tag="qsb")
                            nc.vector.tensor_add(out=qsb[:, :nb, :],
                                                 in0=psq[:, :nb, :],
                                                 in1=xsg[:, sb0:sb0 + nb, :])
                            mt = ep.tile([128, SBC, 128], BF, tag="mt")
                            nc.scalar.activation(out=mt[:, :nb, :],
                                                 in_=qsb[:, :nb, :], func=GELU)
                            P4 = ep.tile([128, SBC, 128], BF, tag="P4")
                            nc.vector.tensor_tensor(
                                out=P4[:, :nb, :],
                                in0=edc_t[:, cc0 + sb0:cc0 + sb0 + nb]
                                    .to_broadcast([128, nb, 128]),
                                in1=wt["iota"][:, None, :]
                                    .to_broadcast([128, nb, 128]),
                                op=EQ)
                            for j in range(nb):
                                cc = cc0 + sb0 + j
                                w, first, last, hh, real = chunk_meta[cc]
                                assert real
                                if first:
                                    active_agg[0] = ap_.tile([128, 128], F32, tag="agg", name="agg_ps")
                                nc.tensor.matmul(out=active_agg[0][:],
                                                 lhsT=P4[:, j, :], rhs=mt[:, j, :],
                                                 start=first, stop=last)
                                if last:
                                    finalize(hh, w, active_agg[0])

            # ---------------- program ----------------
            phases = int(os.environ.get("KERNEL_PHASES", "5"))
            dense_full(0)
            dense_own_l0()
            if phases >= 2:
                edge_phase(0)
            if phases >= 3:
                nc.gpsimd.collective_compute(
                    "AllGather", mybir.AluOpType.bypass,
                    replica_groups=[list(range(NC))],
                    ins=[ag_in[:].opt()], outs=[ag_out[:].opt()])
            if phases >= 4:
                dense_full(1)
                dense_own_l1()
            if phases >= 5:
                edge_phase(1)
            else:
                with tc.tile_pool(name="dbg", bufs=2) as dbp:
                    for w in range(WCNT):
                        dsb = dbp.tile([128, 128], F32, tag="dsb")
                        nc.vector.tensor_copy(out=dsb[:], in_=h_own[:, w * 128:(w + 1) * 128])
                        nc.sync.dma_start(out=t_out[w * 128:(w + 1) * 128, :], in_=dsb[:])

    nc.finalize()
    return nc


_CACHE = {}


def _get_program(L, L0p, chunk_meta):
    key = (L, L0p, tuple(m[:4] for m in chunk_meta))
    if key not in _CACHE:
        _CACHE[key] = _build(L, L0p, chunk_meta)
    return _CACHE[key]


def kernel(**inputs):
    shared, per_core, meta = _prep(inputs)
    nc = _get_program(meta["L"], meta["L0p"], meta["chunk_meta"])
    in_maps = []
    for c in range(NC):
        m = dict(shared)
        m.update(per_core[c])
        in_maps.append(m)
    trace = os.environ.get("KERNEL_TRACE", "0") == "1"
    kw = {}
    if trace:
        kw = dict(trace=True, trace_kwargs={"title": "gnn_mp"})
    res = run_bass_kernel_spmd(nc, in_maps, core_ids=list(range(NC)), **kw)
    if trace and res.exec_time_ns is not None:
        print(f"HW exec time: {res.exec_time_ns} ns")
        if res.instructions_and_trace:
            print("trace:", res.instructions_and_trace[1])
    out = np.concatenate([res.results[c]["out"] for c in range(NC)], axis=0)
    return np.ascontiguousarray(out[:N]).astype(np.float32)



# revision 14
# speedup vs baseline: 1.0037x; 1.0037x over previous
# Trainium2 Bass kernel for the 2-layer GNN message-passing block.
# Self-contained: hardcodes shapes; takes full inputs, shards across 8 cores,
# returns the full [50000, 128] float32 output.
#
# Design (v2):
#  - Layer 0: no gather. Host streams x[src_e] feature-major (xeT); the device
#    computes GELU(W1^T x_e + b1) then accumulates Ws1/We1/xd terms into a
#    feature-major PSUM with stationary-weight N=512 matmuls.
#  - Layer 1: dma_gather (transpose mode -> feature-major) of xs1 from
#    per-epoch AllGather buffers. The layer-1 edge stream is sorted by
#    (src-epoch, dst-window) so epoch-g gathers fire as soon as epoch g's
#    xs1 windows have been AllGathered -> descriptor generation overlaps
#    layer-0 compute.
#  - No dense_full: the AllGather carries xs1 = h1 @ Ws2 directly.
#  - Scatter per chunk via one-hot matmul into PSUM (edge-major); Pt (gather
#    one-hot for the xd term) is the PE-transpose of P4.
import os
import sys

sys.path.insert(0, "/opt/trn_rl_repo")

import numpy as np
import ml_dtypes

import concourse.bacc as bacc
import concourse.tile as tile
from concourse import mybir
from concourse.bass_utils import run_bass_kernel_spmd

BF16 = ml_dtypes.bfloat16

N = 50000
NPAD = 50176
NC = 8
C = NPAD // NC            # 6272 nodes per core
WCNT = C // 128           # 49 windows of 128 nodes
NEP = 4                   # epochs (groups of local windows) for layer-1
EPW = [13, 13, 13, 10]    # windows per epoch
EPW0 = [0, 13, 26, 39]    # first window of each epoch
GOP = int(os.environ.get("KERNEL_GOP", "1024"))  # edges per dma_gather op
GRP = 512                 # edges per compute group (4 chunks)

F32 = mybir.dt.float32
BF = mybir.dt.bfloat16
I16 = mybir.dt.int16


def _bf(x):
    return np.ascontiguousarray(x.astype(BF16))


def _sort_stream(dst_local, sub, nsub, owner, extra_pad_unit):
    """Shared-layout edge stream sort.

    Edges keyed by (owner, sub, win). Returns per-core segment layout shared
    across cores (max counts), satisfying: each (sub, win) segment is a
    multiple of 128 edges (>=128), and each sub block is a multiple of
    extra_pad_unit edges.
    Returns (order, dest, seg info, L, chunk metadata arrays).
    """
    E = dst_local.shape[0]
    win = dst_local // 128
    key = (owner * nsub + sub) * WCNT + win
    order = np.argsort(key, kind="stable")
    ksort = key[order]
    counts_flat = np.bincount(key, minlength=NC * nsub * WCNT)
    counts = counts_flat.reshape(NC, nsub, WCNT)

    nch = np.maximum(1, -(-counts.max(axis=0) // 128)).astype(np.int64)  # [nsub, WCNT]
    seg_len = nch * 128
    sub_len = seg_len.sum(axis=1)                      # [nsub]
    sub_pad = (-sub_len) % extra_pad_unit
    sub_start = np.zeros(nsub, np.int64)
    pos = 0
    for s in range(nsub):
        sub_start[s] = pos
        pos += sub_len[s] + sub_pad[s]
    L = int(pos)

    seg_start = np.zeros((nsub, WCNT), np.int64)
    for s in range(nsub):
        p = sub_start[s]
        for w in range(WCNT):
            seg_start[s, w] = p
            p += seg_len[s, w]

    group_first = np.cumsum(counts_flat) - counts_flat
    within = np.arange(E, dtype=np.int64) - group_first[ksort]
    dest = seg_start[sub[order], win[order]] + within

    nchunks = L // 128
    cw = np.full(nchunks, -1, np.int64)     # window (-1 = pad chunk)
    cf = np.zeros(nchunks, bool)            # first chunk of segment
    cl = np.zeros(nchunks, bool)            # last chunk of segment
    cs = np.zeros(nchunks, np.int64)        # sub index
    for s in range(nsub):
        for w in range(WCNT):
            a = int(seg_start[s, w]) // 128
            n = int(nch[s, w])
            cw[a:a + n] = w
            cf[a] = True
            cl[a + n - 1] = True
            cs[a:a + n] = s
        # pad region of this sub block: mark sub so gather slicing stays
        # within the sub block
        pe = (int(sub_start[s]) + int(sub_len[s])) // 128
        pe2 = pe + int(sub_pad[s]) // 128
        cs[pe:pe2] = s
    return order, dest, L, cw, cf, cl, cs, sub_start, sub_len, sub_pad


def _prep(inputs):
    """Host-side graph partitioning / stream layout. Index+layout prep only."""
    src = np.asarray(inputs["edge_index"][0]).astype(np.int64)
    dst = np.asarray(inputs["edge_index"][1]).astype(np.int64)
    ef = np.asarray(inputs["edge_features"]).astype(np.float32)

    d_owner = dst // C
    dl = dst - d_owner * C

    s_owner = src // C
    s_lw = (src - s_owner * C) // 128
    s_ep = np.minimum(s_lw // 13, 3)

    # ---- layer-0 stream: sorted by (dst window) only ----
    z = np.zeros_like(dst)
    (o0, de0, L0, cw0, cf0, cl0, _, _, _, _) = _sort_stream(
        dl, z, 1, d_owner, GRP)

    # ---- layer-1 stream: sorted by (src epoch, dst window) ----
    (o1, de1, L1, cw1, cf1, cl1, cs1, sub_start1, sub_len1, sub_pad1) = \
        _sort_stream(dl, s_ep, NEP, d_owner, GOP)

    # epoch-buffer row index for every edge (gather idx within its epoch buf)
    ep_rows = np.array([EPW[g] * 128 for g in range(NEP)])
    g = s_ep
    row = s_owner * ep_rows[g] + (s_lw - 13 * g) * 128 + (src - s_owner * C - s_lw * 128)
    assert row.max() < 32768

    x = np.asarray(inputs["x"]).astype(np.float32)
    xpad = np.zeros((NPAD, 128), np.float32)
    xpad[:N] = x
    x_bf = xpad.astype(BF16)

    iota = np.tile(np.arange(128, dtype=np.float32)[None, :], (128, 1))
    ident = np.eye(128, dtype=np.float32)

    shared = {
        "W1": _bf(np.asarray(inputs["ff1_W"], np.float32)),
        "Ws1": _bf(np.asarray(inputs["mp1_Wsrc"], np.float32)),
        "Wd1": _bf(np.asarray(inputs["mp1_Wdst"], np.float32)),
        "We1": _bf(np.asarray(inputs["mp1_We"], np.float32)),
        "Ws2": _bf(np.asarray(inputs["mp2_Wsrc"], np.float32)),
        "Wd2": _bf(np.asarray(inputs["mp2_Wdst"], np.float32)),
        "We2": _bf(np.asarray(inputs["mp2_We"], np.float32)),
        "W3": _bf(np.asarray(inputs["ff2_W"], np.float32)),
        "b1c": np.ascontiguousarray(
            np.asarray(inputs["ff1_b"], np.float32)[:, None]),
        "b1m": np.ascontiguousarray(
            np.tile(np.asarray(inputs["mp1_b"], np.float32)[None, :], (128, 1))),
        "b2m": np.ascontiguousarray(
            np.tile(np.asarray(inputs["mp2_b"], np.float32)[None, :], (128, 1))),
        "b3m": np.ascontiguousarray(
            np.tile(np.asarray(inputs["ff2_b"], np.float32)[None, :], (128, 1))),
        "iota": _bf(iota),
        "identb": _bf(ident),
    }

    per_core = []
    for c in range(NC):
        # layer-0 per-core stream
        m0 = d_owner[o0] == c
        e0_ids = o0[m0]
        dp0 = de0[m0]
        eT0 = np.zeros((64, L0), np.float32)
        eT0[:, dp0] = ef[e0_ids].T
        xeT = np.zeros((128, L0), BF16)
        xeT[:, dp0] = x_bf[src[e0_ids]].T
        edc0 = np.full(L0, -1.0, np.float32)
        edc0[dp0] = dl[e0_ids] % 128

        # layer-1 per-core stream
        m1 = d_owner[o1] == c
        e1_ids = o1[m1]
        dp1 = de1[m1]
        eT1 = np.zeros((64, L1), np.float32)
        eT1[:, dp1] = ef[e1_ids].T
        edc1 = np.full(L1, -1.0, np.float32)
        edc1[dp1] = dl[e1_ids] % 128
        sidx = np.zeros(L1, np.int16)
        sidx[dp1] = row[e1_ids].astype(np.int16)

        per_core.append({
            "eT0": _bf(eT0),
            "xeT": np.ascontiguousarray(xeT),
            "edc0": _bf(np.ascontiguousarray(edc0.reshape(L0 // 128, 128).T)),
            "eT1": _bf(eT1),
            "edc1": _bf(np.ascontiguousarray(edc1.reshape(L1 // 128, 128).T)),
            "srcw": np.ascontiguousarray(
                np.tile(sidx.reshape(L1 // 16, 16).T, (8, 1))),
            "xoT": _bf(xpad[c * C:(c + 1) * C].T),
        })

    meta = dict(
        L0=L0, meta0=list(zip(cw0.tolist(), cf0.tolist(), cl0.tolist())),
        L1=L1, meta1=list(zip(cw1.tolist(), cf1.tolist(), cl1.tolist(),
                              cs1.tolist())),
        ep_start=[int(v) for v in sub_start1],
        ep_end=[int(sub_start1[s] + sub_len1[s] + sub_pad1[s])
                for s in range(NEP)],
    )
    return shared, per_core, meta


def _build(meta):
    """Build the SPMD Bass program (identical for all 8 cores)."""
    L0, meta0 = meta["L0"], meta["meta0"]
    L1, meta1 = meta["L1"], meta["meta1"]
    ep_start, ep_end = meta["ep_start"], meta["ep_end"]

    nc = bacc.Bacc("TRN2", target_bir_lowering=False, debug=False,
                   num_devices=NC, num_swdge_queues=4,
                   dynamic_dma_scratch_size=int(os.environ.get("KERNEL_DDS", "16384")))
    GELU = (mybir.ActivationFunctionType.Identity
            if os.environ.get("KERNEL_SIM_IDENTITY") == "1"
            else mybir.ActivationFunctionType.Gelu_apprx_tanh)
    EQ = mybir.AluOpType.is_equal

    # I/O
    t_xoT = nc.dram_tensor("xoT", [128, C], BF, kind="ExternalInput")
    t_eT0 = nc.dram_tensor("eT0", [64, L0], BF, kind="ExternalInput")
    t_xeT = nc.dram_tensor("xeT", [128, L0], BF, kind="ExternalInput")
    t_edc0 = nc.dram_tensor("edc0", [128, L0 // 128], BF, kind="ExternalInput")
    t_eT1 = nc.dram_tensor("eT1", [64, L1], BF, kind="ExternalInput")
    t_edc1 = nc.dram_tensor("edc1", [128, L1 // 128], BF, kind="ExternalInput")
    t_srcw = nc.dram_tensor("srcw", [128, L1 // 16], I16, kind="ExternalInput")
    wts = {}
    for nm, shape, dt in [
        ("W1", [128, 128], BF), ("Ws1", [128, 128], BF), ("Wd1", [128, 128], BF),
        ("We1", [64, 128], BF), ("Ws2", [128, 128], BF), ("Wd2", [128, 128], BF),
        ("We2", [64, 128], BF), ("W3", [128, 128], BF),
        ("b1c", [128, 1], F32), ("b1m", [128, 128], F32), ("b2m", [128, 128], F32),
        ("b3m", [128, 128], F32), ("iota", [128, 128], BF),
        ("identb", [128, 128], BF),
    ]:
        wts[nm] = nc.dram_tensor(nm, shape, dt, kind="ExternalInput")
    t_out = nc.dram_tensor("out", [C, 128], F32, kind="ExternalOutput")

    with tile.TileContext(nc) as tc:
        with (
            tc.tile_pool(name="persist", bufs=1) as pp,
            tc.tile_pool(name="dram", bufs=1, space="DRAM") as dram,
        ):
            wt = {}
            for nm in ["W1", "Ws1", "Wd1", "We1", "Ws2", "Wd2", "We2", "W3",
                       "b1c", "b1m", "b2m", "b3m", "iota", "identb"]:
                shape = wts[nm].shape
                dt = {"b1c": F32, "b1m": F32, "b2m": F32,
                      "b3m": F32}.get(nm, BF)
                wt[nm] = pp.tile(list(shape), dt, tag=f"w_{nm}", name=f"w_{nm}")
                nc.sync.dma_start(out=wt[nm][:], in_=wts[nm][:])
            edc0_t = pp.tile([128, L0 // 128], BF, tag="edc0")
            nc.sync.dma_start(out=edc0_t[:], in_=t_edc0[:])
            edc1_t = pp.tile([128, L1 // 128], BF, tag="edc1")
            nc.sync.dma_start(out=edc1_t[:], in_=t_edc1[:])
            srcw_t = pp.tile([128, L1 // 16], I16, tag="srcw")
            nc.sync.dma_start(out=srcw_t[:], in_=t_srcw[:])
            h_own = pp.tile([128, C], BF, tag="h_own")
            xd_sb = pp.tile([128, C], BF, tag="xd_sb")
            agg_sb = pp.tile([128, C], F32, tag="agg_sb")

            # per-epoch allgather buffers
            ag_in = [dram.tile([EPW[g] * 128, 128], BF, tag=f"agi{g}",
                               name=f"agi{g}") for g in range(NEP)]
            ag_out = [dram.tile([EPW[g] * 128 * NC, 128], BF, tag=f"ago{g}",
                                name=f"ago{g}", addr_space="Shared")
                      for g in range(NEP)]

            # ---------------- dense phase (own nodes only) ----------------
            def dense_own():
                with (
                    tc.tile_pool(name="dB", bufs=3) as dp,
                    tc.tile_pool(name="dBp", bufs=2, space="PSUM") as dq,
                ):
                    ngrp = (C + 511) // 512
                    for gi in range(ngrp):
                        c0 = gi * 512
                        cn = min(512, C - c0)
                        xt = dp.tile([128, 512], BF, tag="xt")
                        nc.sync.dma_start(out=xt[:, :cn], in_=t_xoT[:, c0:c0 + cn])
                        ps = dq.tile([128, 512], F32, tag="ps")
                        nc.tensor.matmul(out=ps[:, :cn], lhsT=wt["W1"][:],
                                         rhs=xt[:, :cn], start=True, stop=True)
                        hT = dp.tile([128, 512], BF, tag="hT")
                        nc.scalar.activation(out=hT[:, :cn], in_=ps[:, :cn],
                                             func=GELU, bias=wt["b1c"][:])
                        for j in range(cn // 128):
                            lw = c0 + j * 128
                            sl = hT[:, j * 128:(j + 1) * 128]
                            pn = dq.tile([128, 128], F32, tag="pn")
                            nc.tensor.matmul(out=pn[:], lhsT=sl, rhs=wt["identb"][:],
                                             start=True, stop=True)
                            nc.vector.tensor_copy(out=h_own[:, lw:lw + 128], in_=pn[:])
                            pd = dq.tile([128, 128], F32, tag="pd")
                            nc.tensor.matmul(out=pd[:], lhsT=sl, rhs=wt["Wd1"][:],
                                             start=True, stop=True)
                            nc.vector.tensor_add(out=xd_sb[:, lw:lw + 128],
                                                 in0=pd[:], in1=wt["b1m"][:])

            # ---------------- edge phases ----------------
            def edge_phase(layer):
                We = wt["We1"] if layer == 0 else wt["We2"]
                L = L0 if layer == 0 else L1
                meta_ = meta0 if layer == 0 else meta1
                edc_t = edc0_t if layer == 0 else edc1_t
                t_eT = t_eT0 if layer == 0 else t_eT1
                epoch_done = [False] * NEP

                with (
                    tc.tile_pool(name=f"eS{layer}", bufs=4) as ep,
                    tc.tile_pool(name=f"eG{layer}", bufs=3) as gp,
                    tc.tile_pool(name=f"ePH{layer}", bufs=2, space="PSUM") as qh,
                    tc.tile_pool(name=f"ePX{layer}", bufs=2, space="PSUM") as qx,
                    tc.tile_pool(name=f"ePT{layer}", bufs=2, space="PSUM") as qt,
                    tc.tile_pool(name=f"ePA{layer}", bufs=2, space="PSUM") as qa,
                    tc.tile_pool(name=f"eF{layer}", bufs=2) as fp,
                ):
                    active_agg = [None]
                    partial = [False] * WCNT
                    r_gop = nc.gpsimd.to_reg(GOP)

                    def finalize0(w, agg_ps):
                        ws = slice(w * 128, (w + 1) * 128)
                        h1w = fp.tile([128, 128], BF, tag="fh")
                        nc.vector.tensor_add(out=h1w[:], in0=agg_ps[:],
                                             in1=h_own[:, ws])
                        nc.vector.tensor_copy(out=h_own[:, ws], in_=h1w[:])
                        ptr = qt.tile([128, 128], F32, tag="ftp", bufs=1,
                                      name="fptr")
                        nc.tensor.matmul(out=ptr[:], lhsT=h1w[:],
                                         rhs=wt["identb"][:], start=True, stop=True)
                        h1T = fp.tile([128, 128], BF, tag="fh1T")
                        nc.vector.tensor_copy(out=h1T[:], in_=ptr[:])
                        pxd = qt.tile([128, 128], F32, tag="ftp", bufs=1,
                                      name="fpxd")
                        nc.tensor.matmul(out=pxd[:], lhsT=h1T[:], rhs=wt["Wd2"][:],
                                         start=True, stop=True)
                        nc.vector.tensor_add(out=xd_sb[:, ws], in0=pxd[:],
                                             in1=wt["b2m"][:])
                        pxl = qt.tile([128, 128], F32, tag="ftp", bufs=1,
                                      name="fpxl")
                        nc.tensor.matmul(out=pxl[:], lhsT=h1T[:], rhs=wt["Ws2"][:],
                                         start=True, stop=True)
                        xsl = fp.tile([128, 128], BF, tag="fxsl")
                        nc.vector.tensor_copy(out=xsl[:], in_=pxl[:])
                        g = min(w // 13, 3)
                        lw = w - EPW0[g]
                        nc.sync.dma_start(
                            out=ag_in[g][lw * 128:(lw + 1) * 128, :], in_=xsl[:])
                        if w == EPW0[g] + EPW[g] - 1:
                            nc.gpsimd.collective_compute(
                                "AllGather", mybir.AluOpType.bypass,
                                replica_groups=[list(range(NC))],
                                ins=[ag_in[g][:].opt()],
                                outs=[ag_out[g][:].opt()])
                            epoch_done[g] = True

                    def finalize1_seg(w, ep_i, last_ep, agg_ps):
                        ws = slice(w * 128, (w + 1) * 128)
                        if not last_ep:
                            if partial[w]:
                                nc.vector.tensor_add(out=agg_sb[:, ws],
                                                     in0=agg_ps[:],
                                                     in1=agg_sb[:, ws])
                            else:
                                nc.vector.tensor_copy(out=agg_sb[:, ws],
                                                      in_=agg_ps[:])
                                partial[w] = True
                            return
                        t1 = fp.tile([128, 128], F32, tag="f1")
                        if partial[w]:
                            nc.vector.tensor_add(out=t1[:], in0=agg_ps[:],
                                                 in1=agg_sb[:, ws])
                        else:
                            nc.vector.tensor_copy(out=t1[:], in_=agg_ps[:])
                        h2w = fp.tile([128, 128], BF, tag="fh2")
                        nc.vector.tensor_add(out=h2w[:], in0=t1[:],
                                             in1=h_own[:, ws])
                        ptr = qt.tile([128, 128], F32, tag="ftp", bufs=1,
                                      name="fptr2")
                        nc.tensor.matmul(out=ptr[:], lhsT=h2w[:],
                                         rhs=wt["identb"][:], start=True, stop=True)
                        h2T = fp.tile([128, 128], BF, tag="fh2T")
                        nc.vector.tensor_copy(out=h2T[:], in_=ptr[:])
                        po = qt.tile([128, 128], F32, tag="ftp", bufs=1, name="fpo")
                        nc.tensor.matmul(out=po[:], lhsT=h2T[:], rhs=wt["W3"][:],
                                         start=True, stop=True)
                        osb = fp.tile([128, 128], F32, tag="fosb")
                        nc.vector.tensor_add(out=osb[:], in0=po[:], in1=wt["b3m"][:])
                        nc.sync.dma_start(out=t_out[w * 128:(w + 1) * 128, :],
                                          in_=osb[:])

                    # iterate over 512-edge groups
                    xsg_tiles = {}
                    for g0 in range(0, L, GRP):
                        cc0 = g0 // 128
                        chunks = [meta_[cc0 + j] for j in range(4)]
                        if all(ch[0] < 0 for ch in chunks):
                            continue

                        gmode = os.environ.get("KERNEL_GMODE", "t")
                        if layer == 1:
                            # gather (one op per GOP block, covers 2 groups)
                            if g0 % GOP == 0 and gmode != "0":
                                ep_i = next(gg for gg in range(NEP)
                                            if ep_start[gg] <= g0 < ep_end[gg])
                                if gmode == "t":
                                    xsg = gp.tile([128, 1, GOP], BF, tag="xsg")
                                    nc.gpsimd.dma_gather(
                                        xsg[:], ag_out[ep_i][:],
                                        srcw_t[:, g0 // 16:(g0 + GOP) // 16],
                                        GOP, r_gop, 128, transpose=True,
                                        queue_num=(g0 // GOP) % 4)
                                else:
                                    xsg = gp.tile([128, GOP // 128, 128], BF,
                                                  tag="xsg")
                                    nc.gpsimd.dma_gather(
                                        xsg[:], ag_out[ep_i][:],
                                        srcw_t[:, g0 // 16:(g0 + GOP) // 16],
                                        GOP, r_gop, 128, elem_step=128,
                                        queue_num=(g0 // GOP) % 4)
                                xsg_tiles[g0 // GOP] = xsg
                            if gmode != "0":
                                xsg = xsg_tiles[g0 // GOP]
                                xoff = g0 % GOP
                            dmp = os.environ.get("KERNEL_DUMPXSG")
                            if dmp is not None:
                                dbase = int(dmp) * GOP
                                if g0 % GOP == 0 and 0 <= (g0 - dbase) < 6 * GOP:
                                    ds = fp.tile([128, 8, 128], F32, tag="dxs")
                                    nc.vector.tensor_copy(out=ds[:], in_=xsg[:])
                                    nc.sync.dma_start(
                                        out=t_out[g0 - dbase:g0 - dbase + GOP, :]
                                            .rearrange("(c p) f -> p c f", p=128),
                                        in_=ds[:])
                                continue

                        eTt = ep.tile([64, GRP], BF, tag="eTt")
                        nc.sync.dma_start(out=eTt[:], in_=t_eT[:, g0:g0 + GRP])

                        # one-hots
                        P4 = ep.tile([128, 4, 128], BF, tag="P4")
                        nc.vector.tensor_tensor(
                            out=P4[:],
                            in0=edc_t[:, cc0:cc0 + 4]
                                .to_broadcast([128, 4, 128]),
                            in1=wt["iota"][:, None, :].to_broadcast([128, 4, 128]),
                            op=EQ)
                        ptp = qt.tile([128, 4, 128], F32, tag="tps", bufs=1, name="ptp")
                        for j in range(4):
                            nc.tensor.matmul(out=ptp[:, j, :], lhsT=P4[:, j, :],
                                             rhs=wt["identb"][:],
                                             start=True, stop=True)
                        Pt = ep.tile([128, 4, 128], BF, tag="Pt")
                        nc.vector.tensor_copy(out=Pt[:], in_=ptp[:])

                        # feature-major pre-GELU accumulation pxs [f', 512]
                        pxs = qx.tile([128, GRP], F32, tag="pxs")
                        nc.tensor.matmul(out=pxs[:], lhsT=We[:], rhs=eTt[:],
                                         start=True, stop=False)
                        if layer == 0:
                            xet = ep.tile([128, GRP], BF, tag="xet")
                            nc.sync.dma_start(out=xet[:], in_=t_xeT[:, g0:g0 + GRP])
                            ph = qh.tile([128, GRP], F32, tag="ph")
                            nc.tensor.matmul(out=ph[:], lhsT=wt["W1"][:],
                                             rhs=xet[:], start=True, stop=True)
                            heT = ep.tile([128, GRP], BF, tag="heT")
                            nc.scalar.activation(out=heT[:], in_=ph[:],
                                                 func=GELU, bias=wt["b1c"][:])
                            nc.tensor.matmul(out=pxs[:], lhsT=wt["Ws1"][:],
                                             rhs=heT[:], start=False, stop=False)
                        elif gmode == "t":
                            nc.tensor.matmul(
                                out=pxs[:], lhsT=wt["identb"][:],
                                rhs=xsg[:, 0, xoff:xoff + GRP],
                                start=False, stop=False)
                        elif gmode == "n":
                            for j in range(4):
                                nc.tensor.matmul(
                                    out=pxs[:, j * 128:(j + 1) * 128],
                                    lhsT=xsg[:, xoff // 128 + j, :],
                                    rhs=wt["identb"][:],
                                    start=False, stop=False)
                        for j in range(4):
                            w_j = chunks[j][0]
                            if w_j < 0:
                                w_j = 0  # pad chunk: any window; P4 row is zero
                            nc.tensor.matmul(
                                out=pxs[:, j * 128:(j + 1) * 128],
                                lhsT=xd_sb[:, w_j * 128:(w_j + 1) * 128],
                                rhs=Pt[:, j, :], start=False, stop=True)

                        # messages (feature-major), then per-chunk transpose
                        mtT = ep.tile([128, GRP], BF, tag="mtT")
                        nc.scalar.activation(out=mtT[:], in_=pxs[:], func=GELU)
                        mp = qt.tile([128, 4, 128], F32, tag="tps", name="mp",
                                     bufs=1)
                        for j in range(4):
                            nc.tensor.matmul(out=mp[:, j, :],
                                             lhsT=mtT[:, j * 128:(j + 1) * 128],
                                             rhs=wt["identb"][:],
                                             start=True, stop=True)
                        mg = ep.tile([128, 4, 128], BF, tag="mg")
                        nc.vector.tensor_copy(out=mg[:], in_=mp[:])

                        # scatter-accumulate per chunk
                        for j in range(4):
                            ch = meta_[cc0 + j]
                            w = ch[0]
                            if w < 0:
                                continue
                            first, last = ch[1], ch[2]
                            if first:
                                active_agg[0] = qa.tile([128, 128], F32,
                                                        tag="agg", name="agg_ps")
                            nc.tensor.matmul(out=active_agg[0][:],
                                             lhsT=P4[:, j, :], rhs=mg[:, j, :],
                                             start=first, stop=last)
                            if last:
                                if layer == 0:
                                    finalize0(w, active_agg[0])
                                else:
                                    finalize1_seg(w, ch[3], ch[3] == NEP - 1,
                                                  active_agg[0])

            # ---------------- program ----------------
            phases = int(os.environ.get("KERNEL_PHASES", "3"))
            dense_own()
            if phases >= 2:
                edge_phase(0)
            if phases >= 3:
                edge_phase(1)
            else:
                with tc.tile_pool(name="dbg", bufs=2) as dbp:
                    for w in range(WCNT):
                        dsb = dbp.tile([128, 128], F32, tag="dsb")
                        nc.vector.tensor_copy(
                            out=dsb[:], in_=h_own[:, w * 128:(w + 1) * 128])
                        nc.sync.dma_start(
                            out=t_out[w * 128:(w + 1) * 128, :], in_=dsb[:])

    nc.finalize()
    return nc


_CACHE = {}


def _get_program(meta):
    key = (meta["L0"], meta["L1"], tuple(meta["meta0"]), tuple(meta["meta1"]),
           tuple(meta["ep_start"]), tuple(meta["ep_end"]))
    if key not in _CACHE:
        _CACHE[key] = _build(meta)
    return _CACHE[key]


def kernel(**inputs):
    shared, per_core, meta = _prep(inputs)
    nc = _get_program(meta)
    in_maps = []
    for c in range(NC):
        m = dict(shared)
        m.update(per_core[c])
        in_maps.append(m)
    trace = os.environ.get("KERNEL_TRACE", "0") == "1"
    kw = {}
    if trace:
        kw = dict(trace=True, trace_kwargs={"title": "gnn_mp_v2"})
    res = run_bass_kernel_spmd(nc, in_maps, core_ids=list(range(NC)), **kw)
    if trace and res.exec_time_ns is not None:
        print(f"HW exec time: {res.exec_time_ns} ns")
        if res.instructions_and_trace:
            print("trace:", res.instructions_and_trace[1])
    out = np.concatenate([res.results[c]["out"] for c in range(NC)], axis=0)
    return np.ascontiguousarray(out[:N]).astype(np.float32)


# revision 16
# speedup vs baseline: 1.5289x; 1.5233x over previous
# Trainium2 Bass kernel for the 2-layer GNN message-passing block.
# Self-contained: hardcodes shapes; takes full inputs, shards across 8 cores,
# returns the full [50000, 128] float32 output.
#
# Design (v2):
#  - Layer 0: no gather. Host streams x[src_e] feature-major (xeT); the device
#    computes GELU(W1^T x_e + b1) then accumulates Ws1/We1/xd terms into a
#    feature-major PSUM with stationary-weight N=512 matmuls.
#  - Layer 1: dma_gather (transpose mode -> feature-major) of xs1 from
#    per-epoch AllGather buffers. The layer-1 edge stream is sorted by
#    (src-epoch, dst-window) so epoch-g gathers fire as soon as epoch g's
#    xs1 windows have been AllGathered -> descriptor generation overlaps
#    layer-0 compute.
#  - No dense_full: the AllGather carries xs1 = h1 @ Ws2 directly.
#  - Scatter per chunk via one-hot matmul into PSUM (edge-major); Pt (gather
#    one-hot for the xd term) is the PE-transpose of P4.
import os
import sys

sys.path.insert(0, "/opt/trn_rl_repo")

import numpy as np
import ml_dtypes

import concourse.bacc as bacc
import concourse.tile as tile
from concourse import mybir
from concourse.bass_utils import run_bass_kernel_spmd

BF16 = ml_dtypes.bfloat16

N = 50000
NPAD = 50176
NC = 8
C = NPAD // NC            # 6272 nodes per core
WCNT = C // 128           # 49 windows of 128 nodes
NEP = 4                   # epochs (groups of local windows) for layer-1
EPW = [13, 13, 13, 10]    # windows per epoch
EPW0 = [0, 13, 26, 39]    # first window of each epoch
GOP = int(os.environ.get("KERNEL_GOP", "1024"))  # edges per dma_gather op
GRP = 512                 # edges per compute group (4 chunks)

F32 = mybir.dt.float32
BF = mybir.dt.bfloat16
I16 = mybir.dt.int16


def _bf(x):
    return np.ascontiguousarray(x.astype(BF16))


def _sort_stream(dst_local, sub, nsub, owner, extra_pad_unit):
    """Shared-layout edge stream sort.

    Edges keyed by (owner, sub, win). Returns per-core segment layout shared
    across cores (max counts), satisfying: each (sub, win) segment is a
    multiple of 128 edges (>=128), and each sub block is a multiple of
    extra_pad_unit edges.
    Returns (order, dest, seg info, L, chunk metadata arrays).
    """
    E = dst_local.shape[0]
    win = dst_local // 128
    key = (owner * nsub + sub) * WCNT + win
    order = np.argsort(key, kind="stable")
    ksort = key[order]
    counts_flat = np.bincount(key, minlength=NC * nsub * WCNT)
    counts = counts_flat.reshape(NC, nsub, WCNT)

    nch = np.maximum(1, -(-counts.max(axis=0) // 128)).astype(np.int64)  # [nsub, WCNT]
    seg_len = nch * 128
    sub_len = seg_len.sum(axis=1)                      # [nsub]
    sub_pad = (-sub_len) % extra_pad_unit
    sub_start = np.zeros(nsub, np.int64)
    pos = 0
    for s in range(nsub):
        sub_start[s] = pos
        pos += sub_len[s] + sub_pad[s]
    L = int(pos)

    seg_start = np.zeros((nsub, WCNT), np.int64)
    for s in range(nsub):
        p = sub_start[s]
        for w in range(WCNT):
            seg_start[s, w] = p
            p += seg_len[s, w]

    group_first = np.cumsum(counts_flat) - counts_flat
    within = np.arange(E, dtype=np.int64) - group_first[ksort]
    dest = seg_start[sub[order], win[order]] + within

    nchunks = L // 128
    cw = np.full(nchunks, -1, np.int64)     # window (-1 = pad chunk)
    cf = np.zeros(nchunks, bool)            # first chunk of segment
    cl = np.zeros(nchunks, bool)            # last chunk of segment
    cs = np.zeros(nchunks, np.int64)        # sub index
    for s in range(nsub):
        for w in range(WCNT):
            a = int(seg_start[s, w]) // 128
            n = int(nch[s, w])
            cw[a:a + n] = w
            cf[a] = True
            cl[a + n - 1] = True
            cs[a:a + n] = s
        # pad region of this sub block: mark sub so gather slicing stays
        # within the sub block
        pe = (int(sub_start[s]) + int(sub_len[s])) // 128
        pe2 = pe + int(sub_pad[s]) // 128
        cs[pe:pe2] = s
    return order, dest, L, cw, cf, cl, cs, sub_start, sub_len, sub_pad


def _prep(inputs):
    """Host-side graph partitioning / stream layout. Index+layout prep only."""
    src = np.asarray(inputs["edge_index"][0]).astype(np.int64)
    dst = np.asarray(inputs["edge_index"][1]).astype(np.int64)
    ef = np.asarray(inputs["edge_features"]).astype(np.float32)

    d_owner = dst // C
    dl = dst - d_owner * C

    s_owner = src // C
    s_lw = (src - s_owner * C) // 128
    s_ep = np.minimum(s_lw // 13, 3)

    # ---- layer-0 stream: sorted by (dst window) only ----
    z = np.zeros_like(dst)
    (o0, de0, L0, cw0, cf0, cl0, _, _, _, _) = _sort_stream(
        dl, z, 1, d_owner, GRP)

    # ---- layer-1 stream: sorted by (src epoch, dst window) ----
    (o1, de1, L1, cw1, cf1, cl1, cs1, sub_start1, sub_len1, sub_pad1) = \
        _sort_stream(dl, s_ep, NEP, d_owner, GOP)

    # epoch-buffer row index for every edge (gather idx within its epoch buf)
    ep_rows = np.array([EPW[g] * 128 for g in range(NEP)])
    g = s_ep
    row = s_owner * ep_rows[g] + (s_lw - 13 * g) * 128 + (src - s_owner * C - s_lw * 128)
    assert row.max() < 32768

    x = np.asarray(inputs["x"]).astype(np.float32)
    xpad = np.zeros((NPAD, 128), np.float32)
    xpad[:N] = x
    x_bf = xpad.astype(BF16)

    iota = np.tile(np.arange(128, dtype=np.float32)[None, :], (128, 1))
    ident = np.eye(128, dtype=np.float32)

    shared = {
        "W1": _bf(np.asarray(inputs["ff1_W"], np.float32)),
        "Ws1": _bf(np.asarray(inputs["mp1_Wsrc"], np.float32)),
        "Wd1": _bf(np.asarray(inputs["mp1_Wdst"], np.float32)),
        "We1": _bf(np.asarray(inputs["mp1_We"], np.float32)),
        "Ws2": _bf(np.asarray(inputs["mp2_Wsrc"], np.float32)),
        "Wd2": _bf(np.asarray(inputs["mp2_Wdst"], np.float32)),
        "We2": _bf(np.asarray(inputs["mp2_We"], np.float32)),
        "W3": _bf(np.asarray(inputs["ff2_W"], np.float32)),
        "b1c": np.ascontiguousarray(
            np.asarray(inputs["ff1_b"], np.float32)[:, None]),
        "b1m": np.ascontiguousarray(
            np.tile(np.asarray(inputs["mp1_b"], np.float32)[None, :], (128, 1))),
        "b2m": np.ascontiguousarray(
            np.tile(np.asarray(inputs["mp2_b"], np.float32)[None, :], (128, 1))),
        "b3m": np.ascontiguousarray(
            np.tile(np.asarray(inputs["ff2_b"], np.float32)[None, :], (128, 1))),
        "iota": _bf(iota),
        "identb": _bf(ident),
    }

    per_core = []
    for c in range(NC):
        # layer-0 per-core stream
        m0 = d_owner[o0] == c
        e0_ids = o0[m0]
        dp0 = de0[m0]
        eT0 = np.zeros((64, L0), np.float32)
        eT0[:, dp0] = ef[e0_ids].T
        xeT = np.zeros((128, L0), BF16)
        xeT[:, dp0] = x_bf[src[e0_ids]].T
        edc0 = np.full(L0, -1.0, np.float32)
        edc0[dp0] = dl[e0_ids] % 128

        # layer-1 per-core stream
        m1 = d_owner[o1] == c
        e1_ids = o1[m1]
        dp1 = de1[m1]
        eT1 = np.zeros((64, L1), np.float32)
        eT1[:, dp1] = ef[e1_ids].T
        edc1 = np.full(L1, -1.0, np.float32)
        edc1[dp1] = dl[e1_ids] % 128
        sidx = np.zeros(L1, np.int16)
        sidx[dp1] = row[e1_ids].astype(np.int16)

        per_core.append({
            "eT0": _bf(eT0),
            "xeT": np.ascontiguousarray(xeT),
            "edc0": _bf(np.ascontiguousarray(edc0.reshape(L0 // 128, 128).T)),
            "eT1": _bf(eT1),
            "edc1": _bf(np.ascontiguousarray(edc1.reshape(L1 // 128, 128).T)),
            "srcw": np.ascontiguousarray(
                np.tile(sidx.reshape(L1 // 16, 16).T, (8, 1))),
            "xoT": _bf(xpad[c * C:(c + 1) * C].T),
        })

    meta = dict(
        L0=L0, meta0=list(zip(cw0.tolist(), cf0.tolist(), cl0.tolist())),
        L1=L1, meta1=list(zip(cw1.tolist(), cf1.tolist(), cl1.tolist(),
                              cs1.tolist())),
        ep_start=[int(v) for v in sub_start1],
        ep_end=[int(sub_start1[s] + sub_len1[s] + sub_pad1[s])
                for s in range(NEP)],
    )
    return shared, per_core, meta


def _build(meta):
    """Build the SPMD Bass program (identical for all 8 cores)."""
    L0, meta0 = meta["L0"], meta["meta0"]
    L1, meta1 = meta["L1"], meta["meta1"]
    ep_start, ep_end = meta["ep_start"], meta["ep_end"]

    nc = bacc.Bacc("TRN2", target_bir_lowering=False, debug=False,
                   num_devices=NC, num_swdge_queues=4,
                   dynamic_dma_scratch_size=int(os.environ.get("KERNEL_DDS", "16384")))
    GELU = (mybir.ActivationFunctionType.Identity
            if os.environ.get("KERNEL_SIM_IDENTITY") == "1"
            else mybir.ActivationFunctionType.Gelu_apprx_tanh)
    EQ = mybir.AluOpType.is_equal

    # I/O
    t_xoT = nc.dram_tensor("xoT", [128, C], BF, kind="ExternalInput")
    t_eT0 = nc.dram_tensor("eT0", [64, L0], BF, kind="ExternalInput")
    t_xeT = nc.dram_tensor("xeT", [128, L0], BF, kind="ExternalInput")
    t_edc0 = nc.dram_tensor("edc0", [128, L0 // 128], BF, kind="ExternalInput")
    t_eT1 = nc.dram_tensor("eT1", [64, L1], BF, kind="ExternalInput")
    t_edc1 = nc.dram_tensor("edc1", [128, L1 // 128], BF, kind="ExternalInput")
    t_srcw = nc.dram_tensor("srcw", [128, L1 // 16], I16, kind="ExternalInput")
    wts = {}
    for nm, shape, dt in [
        ("W1", [128, 128], BF), ("Ws1", [128, 128], BF), ("Wd1", [128, 128], BF),
        ("We1", [64, 128], BF), ("Ws2", [128, 128], BF), ("Wd2", [128, 128], BF),
        ("We2", [64, 128], BF), ("W3", [128, 128], BF),
        ("b1c", [128, 1], F32), ("b1m", [128, 128], F32), ("b2m", [128, 128], F32),
        ("b3m", [128, 128], F32), ("iota", [128, 128], BF),
        ("identb", [128, 128], BF),
    ]:
        wts[nm] = nc.dram_tensor(nm, shape, dt, kind="ExternalInput")
    t_out = nc.dram_tensor("out", [C, 128], F32, kind="ExternalOutput")

    with tile.TileContext(nc) as tc:
        with (
            tc.tile_pool(name="persist", bufs=1) as pp,
            tc.tile_pool(name="dram", bufs=1, space="DRAM") as dram,
        ):
            wt = {}
            for nm in ["W1", "Ws1", "Wd1", "We1", "Ws2", "Wd2", "We2", "W3",
                       "b1c", "b1m", "b2m", "b3m", "iota", "identb"]:
                shape = wts[nm].shape
                dt = {"b1c": F32, "b1m": F32, "b2m": F32,
                      "b3m": F32}.get(nm, BF)
                wt[nm] = pp.tile(list(shape), dt, tag=f"w_{nm}", name=f"w_{nm}")
                nc.sync.dma_start(out=wt[nm][:], in_=wts[nm][:])
            edc0_t = pp.tile([128, L0 // 128], BF, tag="edc0")
            nc.sync.dma_start(out=edc0_t[:], in_=t_edc0[:])
            edc1_t = pp.tile([128, L1 // 128], BF, tag="edc1")
            nc.sync.dma_start(out=edc1_t[:], in_=t_edc1[:])
            srcw_t = pp.tile([128, L1 // 16], I16, tag="srcw")
            nc.sync.dma_start(out=srcw_t[:], in_=t_srcw[:])
            h_own = pp.tile([128, C], BF, tag="h_own")
            xd_sb = pp.tile([128, C], BF, tag="xd_sb")
            agg_sb = pp.tile([128, C], F32, tag="agg_sb")

            # per-epoch allgather buffers
            ag_in = [dram.tile([EPW[g] * 128, 128], BF, tag=f"agi{g}",
                               name=f"agi{g}") for g in range(NEP)]
            ag_out = [dram.tile([EPW[g] * 128 * NC, 128], BF, tag=f"ago{g}",
                                name=f"ago{g}", addr_space="Shared")
                      for g in range(NEP)]

            # ---------------- dense phase (own nodes only) ----------------
            def dense_own():
                with (
                    tc.tile_pool(name="dB", bufs=3) as dp,
                    tc.tile_pool(name="dBp", bufs=2, space="PSUM") as dq,
                ):
                    ngrp = (C + 511) // 512
                    for gi in range(ngrp):
                        c0 = gi * 512
                        cn = min(512, C - c0)
                        xt = dp.tile([128, 512], BF, tag="xt")
                        nc.sync.dma_start(out=xt[:, :cn], in_=t_xoT[:, c0:c0 + cn])
                        ps = dq.tile([128, 512], F32, tag="ps")
                        nc.tensor.matmul(out=ps[:, :cn], lhsT=wt["W1"][:],
                                         rhs=xt[:, :cn], start=True, stop=True)
                        hT = dp.tile([128, 512], BF, tag="hT")
                        nc.scalar.activation(out=hT[:, :cn], in_=ps[:, :cn],
                                             func=GELU, bias=wt["b1c"][:])
                        for j in range(cn // 128):
                            lw = c0 + j * 128
                            sl = hT[:, j * 128:(j + 1) * 128]
                            pn = dq.tile([128, 128], F32, tag="pn")
                            nc.tensor.matmul(out=pn[:], lhsT=sl, rhs=wt["identb"][:],
                                             start=True, stop=True)
                            nc.vector.tensor_copy(out=h_own[:, lw:lw + 128], in_=pn[:])
                            pd = dq.tile([128, 128], F32, tag="pd")
                            nc.tensor.matmul(out=pd[:], lhsT=sl, rhs=wt["Wd1"][:],
                                             start=True, stop=True)
                            nc.vector.tensor_add(out=xd_sb[:, lw:lw + 128],
                                                 in0=pd[:], in1=wt["b1m"][:])

            # ---------------- edge phases ----------------
            # ---------------- merged edge phases ----------------
            def edge_phases(run_l1):
                gmode = os.environ.get("KERNEL_GMODE", "t")
                xsgb = int(os.environ.get("KERNEL_XSGB", "8"))
                with (
                    tc.tile_pool(name="eS", bufs=4) as ep,
                    tc.tile_pool(name="eG", bufs=xsgb) as gp,
                    tc.tile_pool(name="ePH", bufs=1, space="PSUM") as qh,
                    tc.tile_pool(name="ePX", bufs=2, space="PSUM") as qx,
                    tc.tile_pool(name="ePT", bufs=1, space="PSUM") as qt,
                    tc.tile_pool(name="ePA", bufs=2, space="PSUM") as qa,
                    tc.tile_pool(name="eF", bufs=2) as fp,
                ):
                    active_agg = {0: None, 1: None}
                    partial = [False] * WCNT
                    ag_issued = [False] * NEP
                    w_done = [-1]
                    r_gop = nc.gpsimd.to_reg(GOP)
                    xsg_tiles = {}

                    def finalize0(w, agg_ps):
                        ws = slice(w * 128, (w + 1) * 128)
                        h1w = fp.tile([128, 128], BF, tag="fh")
                        nc.vector.tensor_add(out=h1w[:], in0=agg_ps[:],
                                             in1=h_own[:, ws])
                        nc.vector.tensor_copy(out=h_own[:, ws], in_=h1w[:])
                        ptr = qt.tile([128, 128], F32, tag="ftp", bufs=1,
                                      name="fptr")
                        nc.tensor.matmul(out=ptr[:], lhsT=h1w[:],
                                         rhs=wt["identb"][:], start=True, stop=True)
                        h1T = fp.tile([128, 128], BF, tag="fh1T")
                        nc.vector.tensor_copy(out=h1T[:], in_=ptr[:])
                        pxd = qt.tile([128, 128], F32, tag="ftp", bufs=1,
                                      name="fpxd")
                        nc.tensor.matmul(out=pxd[:], lhsT=h1T[:], rhs=wt["Wd2"][:],
                                         start=True, stop=True)
                        nc.vector.tensor_add(out=xd_sb[:, ws], in0=pxd[:],
                                             in1=wt["b2m"][:])
                        pxl = qt.tile([128, 128], F32, tag="ftp", bufs=1,
                                      name="fpxl")
                        nc.tensor.matmul(out=pxl[:], lhsT=h1T[:], rhs=wt["Ws2"][:],
                                         start=True, stop=True)
                        xsl = fp.tile([128, 128], BF, tag="fxsl")
                        nc.vector.tensor_copy(out=xsl[:], in_=pxl[:])
                        g = min(w // 13, 3)
                        lw = w - EPW0[g]
                        nc.sync.dma_start(
                            out=ag_in[g][lw * 128:(lw + 1) * 128, :], in_=xsl[:])
                        if w == EPW0[g] + EPW[g] - 1:
                            nc.gpsimd.collective_compute(
                                "AllGather", mybir.AluOpType.bypass,
                                replica_groups=[list(range(NC))],
                                ins=[ag_in[g][:].opt()],
                                outs=[ag_out[g][:].opt()])
                            ag_issued[g] = True
                        w_done[0] = w

                    def finalize1_seg(w, last_ep, agg_ps):
                        ws = slice(w * 128, (w + 1) * 128)
                        if not last_ep:
                            if partial[w]:
                                nc.vector.tensor_add(out=agg_sb[:, ws],
                                                     in0=agg_ps[:],
                                                     in1=agg_sb[:, ws])
                            else:
                                nc.vector.tensor_copy(out=agg_sb[:, ws],
                                                      in_=agg_ps[:])
                                partial[w] = True
                            return
                        t1 = fp.tile([128, 128], F32, tag="f1")
                        if partial[w]:
                            nc.vector.tensor_add(out=t1[:], in0=agg_ps[:],
                                                 in1=agg_sb[:, ws])
                        else:
                            nc.vector.tensor_copy(out=t1[:], in_=agg_ps[:])
                        h2w = fp.tile([128, 128], BF, tag="fh2")
                        nc.vector.tensor_add(out=h2w[:], in0=t1[:],
                                             in1=h_own[:, ws])
                        ptr = qt.tile([128, 128], F32, tag="ftp", bufs=1,
                                      name="fptr2")
                        nc.tensor.matmul(out=ptr[:], lhsT=h2w[:],
                                         rhs=wt["identb"][:], start=True, stop=True)
                        h2T = fp.tile([128, 128], BF, tag="fh2T")
                        nc.vector.tensor_copy(out=h2T[:], in_=ptr[:])
                        po = qt.tile([128, 128], F32, tag="ftp", bufs=1, name="fpo")
                        nc.tensor.matmul(out=po[:], lhsT=h2T[:], rhs=wt["W3"][:],
                                         start=True, stop=True)
                        osb = fp.tile([128, 128], F32, tag="fosb")
                        nc.vector.tensor_add(out=osb[:], in0=po[:], in1=wt["b3m"][:])
                        nc.sync.dma_start(out=t_out[w * 128:(w + 1) * 128, :],
                                          in_=osb[:])

                    def emit_group(layer, g0):
                        We = wt["We1"] if layer == 0 else wt["We2"]
                        meta_ = meta0 if layer == 0 else meta1
                        edc_t = edc0_t if layer == 0 else edc1_t
                        t_eT = t_eT0 if layer == 0 else t_eT1
                        cc0 = g0 // 128
                        chunks = [meta_[cc0 + j] for j in range(4)]
                        if all(ch[0] < 0 for ch in chunks):
                            return
                        if layer == 1:
                            if g0 % GOP == 0 and gmode != "0":
                                ep_i = next(gg for gg in range(NEP)
                                            if ep_start[gg] <= g0 < ep_end[gg])
                                if gmode == "t":
                                    xsg = gp.tile([128, 1, GOP], BF, tag="xsg")
                                    nc.gpsimd.dma_gather(
                                        xsg[:], ag_out[ep_i][:],
                                        srcw_t[:, g0 // 16:(g0 + GOP) // 16],
                                        GOP, r_gop, 128, transpose=True,
                                        queue_num=(g0 // GOP) % 4)
                                else:
                                    xsg = gp.tile([128, GOP // 128, 128], BF,
                                                  tag="xsg")
                                    nc.gpsimd.dma_gather(
                                        xsg[:], ag_out[ep_i][:],
                                        srcw_t[:, g0 // 16:(g0 + GOP) // 16],
                                        GOP, r_gop, 128, elem_step=128,
                                        queue_num=(g0 // GOP) % 4)
                                xsg_tiles[g0 // GOP] = xsg
                            if gmode != "0":
                                xsg = xsg_tiles[g0 // GOP]
                                xoff = g0 % GOP

                        eTt = ep.tile([64, GRP], BF, tag="eTt")
                        nc.sync.dma_start(out=eTt[:], in_=t_eT[:, g0:g0 + GRP])

                        P4 = ep.tile([128, 4, 128], BF, tag="P4")
                        nc.vector.tensor_tensor(
                            out=P4[:],
                            in0=edc_t[:, cc0:cc0 + 4]
                                .to_broadcast([128, 4, 128]),
                            in1=wt["iota"][:, None, :].to_broadcast([128, 4, 128]),
                            op=EQ)
                        ptp = qt.tile([128, 4, 128], F32, tag="ptp", bufs=1,
                                      name="ptp")
                        for j in range(4):
                            nc.tensor.matmul(out=ptp[:, j, :], lhsT=P4[:, j, :],
                                             rhs=wt["identb"][:],
                                             start=True, stop=True)
                        Pt = ep.tile([128, 4, 128], BF, tag="Pt")
                        nc.vector.tensor_copy(out=Pt[:], in_=ptp[:])

                        pxs = qx.tile([128, GRP], F32, tag="pxs")
                        nc.tensor.matmul(out=pxs[:], lhsT=We[:], rhs=eTt[:],
                                         start=True, stop=False)
                        if layer == 0:
                            xet = ep.tile([128, GRP], BF, tag="xet")
                            nc.sync.dma_start(out=xet[:], in_=t_xeT[:, g0:g0 + GRP])
                            ph = qh.tile([128, GRP], F32, tag="ph")
                            nc.tensor.matmul(out=ph[:], lhsT=wt["W1"][:],
                                             rhs=xet[:], start=True, stop=True)
                            heT = ep.tile([128, GRP], BF, tag="heT")
                            nc.scalar.activation(out=heT[:], in_=ph[:],
                                                 func=GELU, bias=wt["b1c"][:])
                            nc.tensor.matmul(out=pxs[:], lhsT=wt["Ws1"][:],
                                             rhs=heT[:], start=False, stop=False)
                        elif gmode == "t":
                            nc.tensor.matmul(
                                out=pxs[:], lhsT=wt["identb"][:],
                                rhs=xsg[:, 0, xoff:xoff + GRP],
                                start=False, stop=False)
                        elif gmode == "n":
                            for j in range(4):
                                nc.tensor.matmul(
                                    out=pxs[:, j * 128:(j + 1) * 128],
                                    lhsT=xsg[:, xoff // 128 + j, :],
                                    rhs=wt["identb"][:],
                                    start=False, stop=False)
                        for j in range(4):
                            w_j = chunks[j][0]
                            if w_j < 0:
                                w_j = 0  # pad chunk: P4 row is zero anyway
                            nc.tensor.matmul(
                                out=pxs[:, j * 128:(j + 1) * 128],
                                lhsT=xd_sb[:, w_j * 128:(w_j + 1) * 128],
                                rhs=Pt[:, j, :], start=False, stop=True)

                        mtT = ep.tile([128, GRP], BF, tag="mtT")
                        nc.scalar.activation(out=mtT[:], in_=pxs[:], func=GELU)
                        mp = qt.tile([128, 4, 128], F32, tag="mp", name="mp",
                                     bufs=1)
                        for j in range(4):
                            nc.tensor.matmul(out=mp[:, j, :],
                                             lhsT=mtT[:, j * 128:(j + 1) * 128],
                                             rhs=wt["identb"][:],
                                             start=True, stop=True)
                        mg = ep.tile([128, 4, 128], BF, tag="mg")
                        nc.vector.tensor_copy(out=mg[:], in_=mp[:])

                        for j in range(4):
                            ch = meta_[cc0 + j]
                            w = ch[0]
                            if w < 0:
                                continue
                            first, last = ch[1], ch[2]
                            if first:
                                active_agg[layer] = qa.tile(
                                    [128, 128], F32, tag=f"agg{layer}",
                                    name=f"agg_ps{layer}", bufs=1)
                            nc.tensor.matmul(out=active_agg[layer][:],
                                             lhsT=P4[:, j, :], rhs=mg[:, j, :],
                                             start=first, stop=last)
                            if last:
                                if layer == 0:
                                    finalize0(w, active_agg[layer])
                                else:
                                    finalize1_seg(w, ch[3] == NEP - 1,
                                                  active_agg[layer])

                    # static interleave schedule
                    l1_groups = list(range(0, L1, GRP)) if run_l1 else []
                    l1_maxw = []
                    l1_ep = []
                    for g0 in l1_groups:
                        cc0 = g0 // 128
                        ws_ = [meta1[cc0 + j][0] for j in range(4)
                               if meta1[cc0 + j][0] >= 0]
                        l1_maxw.append(max(ws_) if ws_ else -1)
                        l1_ep.append(next(gg for gg in range(NEP)
                                          if ep_start[gg] <= g0 < ep_end[gg]))
                    l1_i = 0

                    def drain_l1():
                        nonlocal l1_i
                        while l1_i < len(l1_groups):
                            e_ = l1_ep[l1_i]
                            if not ag_issued[e_]:
                                return
                            if l1_maxw[l1_i] > w_done[0]:
                                return
                            emit_group(1, l1_groups[l1_i])
                            l1_i += 1

                    for g0 in range(0, L0, GRP):
                        emit_group(0, g0)
                        if run_l1:
                            drain_l1()
                    if run_l1:
                        w_done[0] = WCNT  # everything finalized
                        drain_l1()
                        assert l1_i == len(l1_groups)

            # ---------------- program ----------------
            phases = int(os.environ.get("KERNEL_PHASES", "3"))
            if phases >= 2:
                dense_own()
                edge_phases(run_l1=(phases >= 3))
            else:
                dense_own()
            if phases < 3:
                with tc.tile_pool(name="dbg", bufs=2) as dbp:
                    for w in range(WCNT):
                        dsb = dbp.tile([128, 128], F32, tag="dsb")
                        nc.vector.tensor_copy(
                            out=dsb[:], in_=h_own[:, w * 128:(w + 1) * 128])
                        nc.sync.dma_start(
                            out=t_out[w * 128:(w + 1) * 128, :], in_=dsb[:])

    nc.finalize()
    return nc


_CACHE = {}


def _get_program(meta):
    key = (meta["L0"], meta["L1"], tuple(meta["meta0"]), tuple(meta["meta1"]),
           tuple(meta["ep_start"]), tuple(meta["ep_end"]))
    if key not in _CACHE:
        _CACHE[key] = _build(meta)
    return _CACHE[key]


def kernel(**inputs):
    shared, per_core, meta = _prep(inputs)
    nc = _get_program(meta)
    in_maps = []
    for c in range(NC):
        m = dict(shared)
        m.update(per_core[c])
        in_maps.append(m)
    trace = os.environ.get("KERNEL_TRACE", "0") == "1"
    kw = {}
    if trace:
        kw = dict(trace=True, trace_kwargs={"title": "gnn_mp_v2"})
    res = run_bass_kernel_spmd(nc, in_maps, core_ids=list(range(NC)), **kw)
    if trace and res.exec_time_ns is not None:
        print(f"HW exec time: {res.exec_time_ns} ns")
        if res.instructions_and_trace:
            print("trace:", res.instructions_and_trace[1])
    out = np.concatenate([res.results[c]["out"] for c in range(NC)], axis=0)
    return np.ascontiguousarray(out[:N]).astype(np.float32)


# revision 17
# speedup vs baseline: 1.5827x; 1.0352x over previous
# Trainium2 Bass kernel for the 2-layer GNN message-passing block.
# Self-contained: hardcodes shapes; takes full inputs, shards across 8 cores,
# returns the full [50000, 128] float32 output.
#
# Design (v2):
#  - Layer 0: no gather. Host streams x[src_e] feature-major (xeT); the device
#    computes GELU(W1^T x_e + b1) then accumulates Ws1/We1/xd terms into a
#    feature-major PSUM with stationary-weight N=512 matmuls.
#  - Layer 1: dma_gather (transpose mode -> feature-major) of xs1 from
#    per-epoch AllGather buffers. The layer-1 edge stream is sorted by
#    (src-epoch, dst-window) so epoch-g gathers fire as soon as epoch g's
#    xs1 windows have been AllGathered -> descriptor generation overlaps
#    layer-0 compute.
#  - No dense_full: the AllGather carries xs1 = h1 @ Ws2 directly.
#  - Scatter per chunk via one-hot matmul into PSUM (edge-major); Pt (gather
#    one-hot for the xd term) is the PE-transpose of P4.
import os
import sys

sys.path.insert(0, "/opt/trn_rl_repo")

import numpy as np
import ml_dtypes

import concourse.bacc as bacc
import concourse.tile as tile
from concourse import mybir
from concourse.bass_utils import run_bass_kernel_spmd

BF16 = ml_dtypes.bfloat16

N = 50000
NPAD = 50176
NC = 8
C = NPAD // NC            # 6272 nodes per core
WCNT = C // 128           # 49 windows of 128 nodes
NEP = 4                   # epochs (groups of local windows) for layer-1
EPW = [13, 13, 13, 10]    # windows per epoch
EPW0 = [0, 13, 26, 39]    # first window of each epoch
GOP = int(os.environ.get("KERNEL_GOP", "1024"))  # edges per dma_gather op
GRP = 512                 # edges per compute group (4 chunks)

F32 = mybir.dt.float32
BF = mybir.dt.bfloat16
I16 = mybir.dt.int16


def _bf(x):
    return np.ascontiguousarray(x.astype(BF16))


def _sort_stream(dst_local, sub, nsub, owner, extra_pad_unit):
    """Shared-layout edge stream sort.

    Edges keyed by (owner, sub, win). Returns per-core segment layout shared
    across cores (max counts), satisfying: each (sub, win) segment is a
    multiple of 128 edges (>=128), and each sub block is a multiple of
    extra_pad_unit edges.
    Returns (order, dest, seg info, L, chunk metadata arrays).
    """
    E = dst_local.shape[0]
    win = dst_local // 128
    key = (owner * nsub + sub) * WCNT + win
    order = np.argsort(key, kind="stable")
    ksort = key[order]
    counts_flat = np.bincount(key, minlength=NC * nsub * WCNT)
    counts = counts_flat.reshape(NC, nsub, WCNT)

    nch = np.maximum(1, -(-counts.max(axis=0) // 128)).astype(np.int64)  # [nsub, WCNT]
    seg_len = nch * 128
    sub_len = seg_len.sum(axis=1)                      # [nsub]
    sub_pad = (-sub_len) % extra_pad_unit
    sub_start = np.zeros(nsub, np.int64)
    pos = 0
    for s in range(nsub):
        sub_start[s] = pos
        pos += sub_len[s] + sub_pad[s]
    L = int(pos)

    seg_start = np.zeros((nsub, WCNT), np.int64)
    for s in range(nsub):
        p = sub_start[s]
        for w in range(WCNT):
            seg_start[s, w] = p
            p += seg_len[s, w]

    group_first = np.cumsum(counts_flat) - counts_flat
    within = np.arange(E, dtype=np.int64) - group_first[ksort]
    dest = seg_start[sub[order], win[order]] + within

    nchunks = L // 128
    cw = np.full(nchunks, -1, np.int64)     # window (-1 = pad chunk)
    cf = np.zeros(nchunks, bool)            # first chunk of segment
    cl = np.zeros(nchunks, bool)            # last chunk of segment
    cs = np.zeros(nchunks, np.int64)        # sub index
    for s in range(nsub):
        for w in range(WCNT):
            a = int(seg_start[s, w]) // 128
            n = int(nch[s, w])
            cw[a:a + n] = w
            cf[a] = True
            cl[a + n - 1] = True
            cs[a:a + n] = s
        # pad region of this sub block: mark sub so gather slicing stays
        # within the sub block
        pe = (int(sub_start[s]) + int(sub_len[s])) // 128
        pe2 = pe + int(sub_pad[s]) // 128
        cs[pe:pe2] = s
    return order, dest, L, cw, cf, cl, cs, sub_start, sub_len, sub_pad


def _prep(inputs):
    """Host-side graph partitioning / stream layout. Index+layout prep only."""
    src = np.asarray(inputs["edge_index"][0]).astype(np.int64)
    dst = np.asarray(inputs["edge_index"][1]).astype(np.int64)
    ef = np.asarray(inputs["edge_features"]).astype(np.float32)

    d_owner = dst // C
    dl = dst - d_owner * C

    s_owner = src // C
    s_lw = (src - s_owner * C) // 128
    s_ep = np.minimum(s_lw // 13, 3)

    # ---- layer-0 stream: sorted by (dst window) only ----
    z = np.zeros_like(dst)
    (o0, de0, L0, cw0, cf0, cl0, _, _, _, _) = _sort_stream(
        dl, z, 1, d_owner, GRP)

    # ---- layer-1 stream: sorted by (src epoch, dst window) ----
    (o1, de1, L1, cw1, cf1, cl1, cs1, sub_start1, sub_len1, sub_pad1) = \
        _sort_stream(dl, s_ep, NEP, d_owner, GOP)

    # epoch-buffer row index for every edge (gather idx within its epoch buf)
    ep_rows = np.array([EPW[g] * 128 for g in range(NEP)])
    g = s_ep
    row = s_owner * ep_rows[g] + (s_lw - 13 * g) * 128 + (src - s_owner * C - s_lw * 128)
    assert row.max() < 32768

    x = np.asarray(inputs["x"]).astype(np.float32)
    xpad = np.zeros((NPAD, 128), np.float32)
    xpad[:N] = x
    x_bf = xpad.astype(BF16)

    iota = np.tile(np.arange(128, dtype=np.float32)[None, :], (128, 1))
    ident = np.eye(128, dtype=np.float32)

    shared = {
        "W1": _bf(np.asarray(inputs["ff1_W"], np.float32)),
        "Ws1": _bf(np.asarray(inputs["mp1_Wsrc"], np.float32)),
        "Wd1": _bf(np.asarray(inputs["mp1_Wdst"], np.float32)),
        "We1": _bf(np.asarray(inputs["mp1_We"], np.float32)),
        "Ws2": _bf(np.asarray(inputs["mp2_Wsrc"], np.float32)),
        "Wd2": _bf(np.asarray(inputs["mp2_Wdst"], np.float32)),
        "We2": _bf(np.asarray(inputs["mp2_We"], np.float32)),
        "W3": _bf(np.asarray(inputs["ff2_W"], np.float32)),
        "b1c": np.ascontiguousarray(
            np.asarray(inputs["ff1_b"], np.float32)[:, None]),
        "b1m": np.ascontiguousarray(
            np.tile(np.asarray(inputs["mp1_b"], np.float32)[None, :], (128, 1))),
        "b2m": np.ascontiguousarray(
            np.tile(np.asarray(inputs["mp2_b"], np.float32)[None, :], (128, 1))),
        "b3m": np.ascontiguousarray(
            np.tile(np.asarray(inputs["ff2_b"], np.float32)[None, :], (128, 1))),
        "iota": _bf(iota),
        "identb": _bf(ident),
    }

    per_core = []
    for c in range(NC):
        # layer-0 per-core stream
        m0 = d_owner[o0] == c
        e0_ids = o0[m0]
        dp0 = de0[m0]
        eT0 = np.zeros((64, L0), np.float32)
        eT0[:, dp0] = ef[e0_ids].T
        xeT = np.zeros((128, L0), BF16)
        xeT[:, dp0] = x_bf[src[e0_ids]].T
        edc0 = np.full(L0, -1.0, np.float32)
        edc0[dp0] = dl[e0_ids] % 128

        # layer-1 per-core stream
        m1 = d_owner[o1] == c
        e1_ids = o1[m1]
        dp1 = de1[m1]
        eT1 = np.zeros((64, L1), np.float32)
        eT1[:, dp1] = ef[e1_ids].T
        edc1 = np.full(L1, -1.0, np.float32)
        edc1[dp1] = dl[e1_ids] % 128
        sidx = np.zeros(L1, np.int16)
        sidx[dp1] = row[e1_ids].astype(np.int16)

        per_core.append({
            "eT0": _bf(eT0),
            "xeT": np.ascontiguousarray(xeT),
            "edc0": _bf(np.ascontiguousarray(edc0.reshape(L0 // 128, 128).T)),
            "eT1": _bf(eT1),
            "edc1": _bf(np.ascontiguousarray(edc1.reshape(L1 // 128, 128).T)),
            "srcw": np.ascontiguousarray(
                np.tile(sidx.reshape(L1 // 16, 16).T, (8, 1))),
            "xoT": _bf(xpad[c * C:(c + 1) * C].T),
        })

    meta = dict(
        L0=L0, meta0=list(zip(cw0.tolist(), cf0.tolist(), cl0.tolist())),
        L1=L1, meta1=list(zip(cw1.tolist(), cf1.tolist(), cl1.tolist(),
                              cs1.tolist())),
        ep_start=[int(v) for v in sub_start1],
        ep_end=[int(sub_start1[s] + sub_len1[s] + sub_pad1[s])
                for s in range(NEP)],
    )
    return shared, per_core, meta


def _build(meta):
    """Build the SPMD Bass program (identical for all 8 cores)."""
    L0, meta0 = meta["L0"], meta["meta0"]
    L1, meta1 = meta["L1"], meta["meta1"]
    ep_start, ep_end = meta["ep_start"], meta["ep_end"]

    nc = bacc.Bacc("TRN2", target_bir_lowering=False, debug=False,
                   num_devices=NC, num_swdge_queues=4,
                   dynamic_dma_scratch_size=int(os.environ.get("KERNEL_DDS", "16384")))
    GELU = (mybir.ActivationFunctionType.Identity
            if os.environ.get("KERNEL_SIM_IDENTITY") == "1"
            else mybir.ActivationFunctionType.Gelu_apprx_tanh)
    EQ = mybir.AluOpType.is_equal

    # I/O
    t_xoT = nc.dram_tensor("xoT", [128, C], BF, kind="ExternalInput")
    t_eT0 = nc.dram_tensor("eT0", [64, L0], BF, kind="ExternalInput")
    t_xeT = nc.dram_tensor("xeT", [128, L0], BF, kind="ExternalInput")
    t_edc0 = nc.dram_tensor("edc0", [128, L0 // 128], BF, kind="ExternalInput")
    t_eT1 = nc.dram_tensor("eT1", [64, L1], BF, kind="ExternalInput")
    t_edc1 = nc.dram_tensor("edc1", [128, L1 // 128], BF, kind="ExternalInput")
    t_srcw = nc.dram_tensor("srcw", [128, L1 // 16], I16, kind="ExternalInput")
    wts = {}
    for nm, shape, dt in [
        ("W1", [128, 128], BF), ("Ws1", [128, 128], BF), ("Wd1", [128, 128], BF),
        ("We1", [64, 128], BF), ("Ws2", [128, 128], BF), ("Wd2", [128, 128], BF),
        ("We2", [64, 128], BF), ("W3", [128, 128], BF),
        ("b1c", [128, 1], F32), ("b1m", [128, 128], F32), ("b2m", [128, 128], F32),
        ("b3m", [128, 128], F32), ("iota", [128, 128], BF),
        ("identb", [128, 128], BF),
    ]:
        wts[nm] = nc.dram_tensor(nm, shape, dt, kind="ExternalInput")
    t_out = nc.dram_tensor("out", [C, 128], F32, kind="ExternalOutput")

    with tile.TileContext(nc) as tc:
        with (
            tc.tile_pool(name="persist", bufs=1) as pp,
            tc.tile_pool(name="dram", bufs=1, space="DRAM") as dram,
        ):
            wt = {}
            for nm in ["W1", "Ws1", "Wd1", "We1", "Ws2", "Wd2", "We2", "W3",
                       "b1c", "b1m", "b2m", "b3m", "iota", "identb"]:
                shape = wts[nm].shape
                dt = {"b1c": F32, "b1m": F32, "b2m": F32,
                      "b3m": F32}.get(nm, BF)
                wt[nm] = pp.tile(list(shape), dt, tag=f"w_{nm}", name=f"w_{nm}")
                nc.sync.dma_start(out=wt[nm][:], in_=wts[nm][:])
            edc0_t = pp.tile([128, L0 // 128], BF, tag="edc0")
            nc.sync.dma_start(out=edc0_t[:], in_=t_edc0[:])
            edc1_t = pp.tile([128, L1 // 128], BF, tag="edc1")
            nc.sync.dma_start(out=edc1_t[:], in_=t_edc1[:])
            srcw_t = pp.tile([128, L1 // 16], I16, tag="srcw")
            nc.sync.dma_start(out=srcw_t[:], in_=t_srcw[:])
            h_own = pp.tile([128, C], BF, tag="h_own")
            xd_sb = pp.tile([128, C], BF, tag="xd_sb")
            agg_sb = pp.tile([128, C], F32, tag="agg_sb")

            # per-epoch allgather buffers
            ag_in = [dram.tile([EPW[g] * 128, 128], BF, tag=f"agi{g}",
                               name=f"agi{g}") for g in range(NEP)]
            ag_out = [dram.tile([EPW[g] * 128 * NC, 128], BF, tag=f"ago{g}",
                                name=f"ago{g}", addr_space="Shared")
                      for g in range(NEP)]

            # ---------------- dense phase (own nodes only) ----------------
            def dense_own():
                with (
                    tc.tile_pool(name="dB", bufs=3) as dp,
                    tc.tile_pool(name="dBp", bufs=2, space="PSUM") as dq,
                ):
                    ngrp = (C + 511) // 512
                    for gi in range(ngrp):
                        c0 = gi * 512
                        cn = min(512, C - c0)
                        xt = dp.tile([128, 512], BF, tag="xt")
                        nc.sync.dma_start(out=xt[:, :cn], in_=t_xoT[:, c0:c0 + cn])
                        ps = dq.tile([128, 512], F32, tag="ps")
                        nc.tensor.matmul(out=ps[:, :cn], lhsT=wt["W1"][:],
                                         rhs=xt[:, :cn], start=True, stop=True)
                        hT = dp.tile([128, 512], BF, tag="hT")
                        nc.scalar.activation(out=hT[:, :cn], in_=ps[:, :cn],
                                             func=GELU, bias=wt["b1c"][:])
                        for j in range(cn // 128):
                            lw = c0 + j * 128
                            sl = hT[:, j * 128:(j + 1) * 128]
                            pn = dq.tile([128, 128], F32, tag="pn")
                            nc.tensor.matmul(out=pn[:], lhsT=sl, rhs=wt["identb"][:],
                                             start=True, stop=True)
                            nc.vector.tensor_copy(out=h_own[:, lw:lw + 128], in_=pn[:])
                            pd = dq.tile([128, 128], F32, tag="pd")
                            nc.tensor.matmul(out=pd[:], lhsT=sl, rhs=wt["Wd1"][:],
                                             start=True, stop=True)
                            nc.vector.tensor_add(out=xd_sb[:, lw:lw + 128],
                                                 in0=pd[:], in1=wt["b1m"][:])

            # ---------------- edge phases ----------------
            # ---------------- merged edge phases ----------------
            def edge_phases(run_l1):
                gmode = os.environ.get("KERNEL_GMODE", "t")
                xsgb = int(os.environ.get("KERNEL_XSGB", "16"))
                with (
                    tc.tile_pool(name="eS", bufs=4) as ep,
                    tc.tile_pool(name="eG", bufs=xsgb) as gp,
                    tc.tile_pool(name="ePH", bufs=1, space="PSUM") as qh,
                    tc.tile_pool(name="ePX", bufs=2, space="PSUM") as qx,
                    tc.tile_pool(name="ePT", bufs=1, space="PSUM") as qt,
                    tc.tile_pool(name="ePA", bufs=2, space="PSUM") as qa,
                    tc.tile_pool(name="eF", bufs=2) as fp,
                ):
                    active_agg = {0: None, 1: None}
                    partial = [False] * WCNT
                    ag_issued = [False] * NEP
                    ag_at = [0] * NEP
                    l0_prog = [0]
                    lead = int(os.environ.get("KERNEL_LEAD", "16"))
                    w_done = [-1]
                    r_gop = nc.gpsimd.to_reg(GOP)
                    xsg_tiles = {}

                    def finalize0(w, agg_ps):
                        ws = slice(w * 128, (w + 1) * 128)
                        h1w = fp.tile([128, 128], BF, tag="fh")
                        nc.vector.tensor_add(out=h1w[:], in0=agg_ps[:],
                                             in1=h_own[:, ws])
                        nc.vector.tensor_copy(out=h_own[:, ws], in_=h1w[:])
                        ptr = qt.tile([128, 128], F32, tag="ftp", bufs=1,
                                      name="fptr")
                        nc.tensor.matmul(out=ptr[:], lhsT=h1w[:],
                                         rhs=wt["identb"][:], start=True, stop=True)
                        h1T = fp.tile([128, 128], BF, tag="fh1T")
                        nc.vector.tensor_copy(out=h1T[:], in_=ptr[:])
                        pxd = qt.tile([128, 128], F32, tag="ftp", bufs=1,
                                      name="fpxd")
                        nc.tensor.matmul(out=pxd[:], lhsT=h1T[:], rhs=wt["Wd2"][:],
                                         start=True, stop=True)
                        nc.vector.tensor_add(out=xd_sb[:, ws], in0=pxd[:],
                                             in1=wt["b2m"][:])
                        pxl = qt.tile([128, 128], F32, tag="ftp", bufs=1,
                                      name="fpxl")
                        nc.tensor.matmul(out=pxl[:], lhsT=h1T[:], rhs=wt["Ws2"][:],
                                         start=True, stop=True)
                        xsl = fp.tile([128, 128], BF, tag="fxsl")
                        nc.vector.tensor_copy(out=xsl[:], in_=pxl[:])
                        g = min(w // 13, 3)
                        lw = w - EPW0[g]
                        nc.sync.dma_start(
                            out=ag_in[g][lw * 128:(lw + 1) * 128, :], in_=xsl[:])
                        if w == EPW0[g] + EPW[g] - 1:
                            nc.gpsimd.collective_compute(
                                "AllGather", mybir.AluOpType.bypass,
                                replica_groups=[list(range(NC))],
                                ins=[ag_in[g][:].opt()],
                                outs=[ag_out[g][:].opt()])
                            ag_issued[g] = True
                            ag_at[g] = l0_prog[0]
                        w_done[0] = w

                    def finalize1_seg(w, last_ep, agg_ps):
                        ws = slice(w * 128, (w + 1) * 128)
                        if not last_ep:
                            if partial[w]:
                                nc.vector.tensor_add(out=agg_sb[:, ws],
                                                     in0=agg_ps[:],
                                                     in1=agg_sb[:, ws])
                            else:
                                nc.vector.tensor_copy(out=agg_sb[:, ws],
                                                      in_=agg_ps[:])
                                partial[w] = True
                            return
                        t1 = fp.tile([128, 128], F32, tag="f1")
                        if partial[w]:
                            nc.vector.tensor_add(out=t1[:], in0=agg_ps[:],
                                                 in1=agg_sb[:, ws])
                        else:
                            nc.vector.tensor_copy(out=t1[:], in_=agg_ps[:])
                        h2w = fp.tile([128, 128], BF, tag="fh2")
                        nc.vector.tensor_add(out=h2w[:], in0=t1[:],
                                             in1=h_own[:, ws])
                        ptr = qt.tile([128, 128], F32, tag="ftp", bufs=1,
                                      name="fptr2")
                        nc.tensor.matmul(out=ptr[:], lhsT=h2w[:],
                                         rhs=wt["identb"][:], start=True, stop=True)
                        h2T = fp.tile([128, 128], BF, tag="fh2T")
                        nc.vector.tensor_copy(out=h2T[:], in_=ptr[:])
                        po = qt.tile([128, 128], F32, tag="ftp", bufs=1, name="fpo")
                        nc.tensor.matmul(out=po[:], lhsT=h2T[:], rhs=wt["W3"][:],
                                         start=True, stop=True)
                        osb = fp.tile([128, 128], F32, tag="fosb")
                        nc.vector.tensor_add(out=osb[:], in0=po[:], in1=wt["b3m"][:])
                        nc.sync.dma_start(out=t_out[w * 128:(w + 1) * 128, :],
                                          in_=osb[:])

                    def emit_group(layer, g0):
                        We = wt["We1"] if layer == 0 else wt["We2"]
                        meta_ = meta0 if layer == 0 else meta1
                        edc_t = edc0_t if layer == 0 else edc1_t
                        t_eT = t_eT0 if layer == 0 else t_eT1
                        cc0 = g0 // 128
                        chunks = [meta_[cc0 + j] for j in range(4)]
                        if all(ch[0] < 0 for ch in chunks):
                            return
                        if layer == 1:
                            if g0 % GOP == 0 and gmode != "0":
                                ep_i = next(gg for gg in range(NEP)
                                            if ep_start[gg] <= g0 < ep_end[gg])
                                if gmode == "t":
                                    xsg = gp.tile([128, 1, GOP], BF, tag="xsg")
                                    nc.gpsimd.dma_gather(
                                        xsg[:], ag_out[ep_i][:],
                                        srcw_t[:, g0 // 16:(g0 + GOP) // 16],
                                        GOP, r_gop, 128, transpose=True,
                                        queue_num=(g0 // GOP) % 4)
                                else:
                                    xsg = gp.tile([128, GOP // 128, 128], BF,
                                                  tag="xsg")
                                    nc.gpsimd.dma_gather(
                                        xsg[:], ag_out[ep_i][:],
                                        srcw_t[:, g0 // 16:(g0 + GOP) // 16],
                                        GOP, r_gop, 128, elem_step=128,
                                        queue_num=(g0 // GOP) % 4)
                                xsg_tiles[g0 // GOP] = xsg
                            if gmode != "0":
                                xsg = xsg_tiles[g0 // GOP]
                                xoff = g0 % GOP

                        eTt = ep.tile([64, GRP], BF, tag="eTt")
                        nc.sync.dma_start(out=eTt[:], in_=t_eT[:, g0:g0 + GRP])

                        P4 = ep.tile([128, 4, 128], BF, tag="P4")
                        nc.vector.tensor_tensor(
                            out=P4[:],
                            in0=wt["iota"][:, None, :].to_broadcast([128, 4, 128]),
                            in1=edc_t[:, cc0:cc0 + 4]
                                .to_broadcast([128, 4, 128]),
                            op=EQ)
                        ptp = qt.tile([128, 4, 128], F32, tag="ptp", bufs=1,
                                      name="ptp")
                        for j in range(4):
                            nc.tensor.matmul(out=ptp[:, j, :], lhsT=P4[:, j, :],
                                             rhs=wt["identb"][:],
                                             start=True, stop=True)
                        Pt = ep.tile([128, 4, 128], BF, tag="Pt")
                        nc.vector.tensor_copy(out=Pt[:], in_=ptp[:])

                        pxs = qx.tile([128, GRP], F32, tag="pxs")
                        nc.tensor.matmul(out=pxs[:], lhsT=We[:], rhs=eTt[:],
                                         start=True, stop=False)
                        if layer == 0:
                            xet = ep.tile([128, GRP], BF, tag="xet")
                            nc.sync.dma_start(out=xet[:], in_=t_xeT[:, g0:g0 + GRP])
                            ph = qh.tile([128, GRP], F32, tag="ph")
                            nc.tensor.matmul(out=ph[:], lhsT=wt["W1"][:],
                                             rhs=xet[:], start=True, stop=True)
                            heT = ep.tile([128, GRP], BF, tag="heT")
                            nc.scalar.activation(out=heT[:], in_=ph[:],
                                                 func=GELU, bias=wt["b1c"][:])
                            nc.tensor.matmul(out=pxs[:], lhsT=wt["Ws1"][:],
                                             rhs=heT[:], start=False, stop=False)
                        elif gmode == "t":
                            nc.tensor.matmul(
                                out=pxs[:], lhsT=wt["identb"][:],
                                rhs=xsg[:, 0, xoff:xoff + GRP],
                                start=False, stop=False)
                        elif gmode == "n":
                            for j in range(4):
                                nc.tensor.matmul(
                                    out=pxs[:, j * 128:(j + 1) * 128],
                                    lhsT=xsg[:, xoff // 128 + j, :],
                                    rhs=wt["identb"][:],
                                    start=False, stop=False)
                        for j in range(4):
                            w_j = chunks[j][0]
                            if w_j < 0:
                                w_j = 0  # pad chunk: P4 row is zero anyway
                            nc.tensor.matmul(
                                out=pxs[:, j * 128:(j + 1) * 128],
                                lhsT=xd_sb[:, w_j * 128:(w_j + 1) * 128],
                                rhs=Pt[:, j, :], start=False, stop=True)

                        mtT = ep.tile([128, GRP], BF, tag="mtT")
                        nc.scalar.activation(out=mtT[:], in_=pxs[:], func=GELU)
                        mp = qt.tile([128, 4, 128], F32, tag="mp", name="mp",
                                     bufs=1)
                        for j in range(4):
                            nc.tensor.matmul(out=mp[:, j, :],
                                             lhsT=mtT[:, j * 128:(j + 1) * 128],
                                             rhs=wt["identb"][:],
                                             start=True, stop=True)
                        mg = ep.tile([128, 4, 128], BF, tag="mg")
                        if os.environ.get("KERNEL_MGACT", "1") == "1":
                            nc.scalar.activation(
                                out=mg[:], in_=mp[:],
                                func=mybir.ActivationFunctionType.Identity)
                        else:
                            nc.vector.tensor_copy(out=mg[:], in_=mp[:])

                        for j in range(4):
                            ch = meta_[cc0 + j]
                            w = ch[0]
                            if w < 0:
                                continue
                            first, last = ch[1], ch[2]
                            if first:
                                active_agg[layer] = qa.tile(
                                    [128, 128], F32, tag=f"agg{layer}",
                                    name=f"agg_ps{layer}", bufs=1)
                            nc.tensor.matmul(out=active_agg[layer][:],
                                             lhsT=P4[:, j, :], rhs=mg[:, j, :],
                                             start=first, stop=last)
                            if last:
                                if layer == 0:
                                    finalize0(w, active_agg[layer])
                                else:
                                    finalize1_seg(w, ch[3] == NEP - 1,
                                                  active_agg[layer])

                    # static interleave schedule
                    l1_groups = list(range(0, L1, GRP)) if run_l1 else []
                    l1_maxw = []
                    l1_ep = []
                    for g0 in l1_groups:
                        cc0 = g0 // 128
                        ws_ = [meta1[cc0 + j][0] for j in range(4)
                               if meta1[cc0 + j][0] >= 0]
                        l1_maxw.append(max(ws_) if ws_ else -1)
                        l1_ep.append(next(gg for gg in range(NEP)
                                          if ep_start[gg] <= g0 < ep_end[gg]))
                    l1_i = 0

                    def drain_l1(final=False):
                        nonlocal l1_i
                        while l1_i < len(l1_groups):
                            e_ = l1_ep[l1_i]
                            if not ag_issued[e_]:
                                return
                            if not final and l0_prog[0] - ag_at[e_] < lead:
                                return
                            if l1_maxw[l1_i] > w_done[0]:
                                return
                            emit_group(1, l1_groups[l1_i])
                            l1_i += 1

                    for g0 in range(0, L0, GRP):
                        emit_group(0, g0)
                        l0_prog[0] += 1
                        if run_l1:
                            drain_l1()
                    if run_l1:
                        w_done[0] = WCNT  # everything finalized
                        drain_l1(final=True)
                        assert l1_i == len(l1_groups)

            # ---------------- program ----------------
            phases = int(os.environ.get("KERNEL_PHASES", "3"))
            if phases >= 2:
                dense_own()
                edge_phases(run_l1=(phases >= 3))
            else:
                dense_own()
            if phases < 3:
                with tc.tile_pool(name="dbg", bufs=2) as dbp:
                    for w in range(WCNT):
                        dsb = dbp.tile([128, 128], F32, tag="dsb")
                        nc.vector.tensor_copy(
                            out=dsb[:], in_=h_own[:, w * 128:(w + 1) * 128])
                        nc.sync.dma_start(
                            out=t_out[w * 128:(w + 1) * 128, :], in_=dsb[:])

    nc.finalize()
    return nc


_CACHE = {}


def _get_program(meta):
    key = (meta["L0"], meta["L1"], tuple(meta["meta0"]), tuple(meta["meta1"]),
           tuple(meta["ep_start"]), tuple(meta["ep_end"]))
    if key not in _CACHE:
        _CACHE[key] = _build(meta)
    return _CACHE[key]


def kernel(**inputs):
    shared, per_core, meta = _prep(inputs)
    nc = _get_program(meta)
    in_maps = []
    for c in range(NC):
        m = dict(shared)
        m.update(per_core[c])
        in_maps.append(m)
    trace = os.environ.get("KERNEL_TRACE", "0") == "1"
    kw = {}
    if trace:
        kw = dict(trace=True, trace_kwargs={"title": "gnn_mp_v2"})
    res = run_bass_kernel_spmd(nc, in_maps, core_ids=list(range(NC)), **kw)
    if trace and res.exec_time_ns is not None:
        print(f"HW exec time: {res.exec_time_ns} ns")
        if res.instructions_and_trace:
            print("trace:", res.instructions_and_trace[1])
    out = np.concatenate([res.results[c]["out"] for c in range(NC)], axis=0)
    return np.ascontiguousarray(out[:N]).astype(np.float32)
